# revision 56
# baseline (speedup 1.0000x reference)
"""Trainium2 Bass kernel for HIVNet GCN message passing (8-core SPMD).

v8 strategy (baseline 2.29ms -> v6 pure-dense 826us -> v7 DoubleRow 638us):
  - Pad N=10000 nodes to 10240 = 80 chunks x 128; core c owns 10 dst-blocks
    (global chunks c*10..c*10+9).
  - Per layer: hws = (h @ W[l])*nrm*32 on the owned shard, cast fp8e4m3,
    AllGather the partition-major table in two halves; aggregation is pure
    dense one-hot adjacency on TensorE using fp8 DoubleRow matmuls
    (both operands fp8, contraction 256/instruction, 2x bf16 throughput).
    The x32 table scale keeps hws out of fp8 subnormals; the dst-side norm
    carries the 1/32.
  - h lives TRANSPOSED (h^T: H on partitions, nodes on free dim):
      * the next-layer GEMM consumes h^T directly as lhsT (no transposes),
      * BN apply is one fused per-partition tensor_scalar (t^T*a + c) + relu,
      * t^T transposes run inside the BN AllReduce window (Tensor idle),
      * BN scale/shift column-ized via 4 tiny matmuls (no 128-row bcast).
  - BN stats: fused sum||sumsq reduce, 32-row replicate, Shared-output
    AllReduce; warmup AllGather at t=0 absorbs comms cold-start skew.
  - Readout: transpose h back per block (last layer only), one-hot pool
    matmuls accumulated in SBUF, 257-row AllReduce, redundant 3-layer MLP.
"""

import os
import sys

sys.path.insert(0, "/opt/trn_rl_repo")

from contextlib import ExitStack

import numpy as np
import ml_dtypes

from concourse import bass, mybir, bacc, tile, library_config
from concourse.bass_utils import run_bass_kernel_spmd
from concourse.masks import make_identity

NCORE = 8
P = 128
H = 256
L = 4
NF = 9
G = 256
N = 10000
BPC = 10                # dst blocks per core
NPC = BPC * P           # 1280 nodes per core
NPAD = NCORE * NPC      # 10240
NCHUNK = NPAD // P      # 80 src chunks
HB = BPC // 2           # blocks per AllGather half
BN_EPS = 1e-5
TSCALE = 32.0           # fp8 table scale

f32 = mybir.dt.float32
bf16 = mybir.dt.bfloat16
f8 = mybir.dt.float8e4
bfnp = ml_dtypes.bfloat16

FT = mybir.ActivationFunctionType
OP = mybir.AluOpType
DR = mybir.MatmulPerfMode.DoubleRow

_compiled = {}

NSEG = 5                # AllGather pieces per layer (2 blocks each)
BPS = BPC // NSEG       # blocks per gather piece
CPS = NCORE * BPS       # chunks per gather piece (16)

# chunk consumption order: fifth-major (blocks {2s,2s+1} of every core form
# gather piece s), so dense-chain segment s can start as soon as piece s
# lands; within a piece, core-major ascending = the gathered tab layout.
CHUNK_ORDER = [g for s in range(NSEG) for g in range(NCHUNK)
               if g % BPC in (2 * s, 2 * s + 1)]


# --------------------------------------------------------------------------
# host-side structural preprocessing
# --------------------------------------------------------------------------

def _preprocess(x, edge_index, batch_ids, emb, W, gamma, beta,
                mlp_W1, mlp_b1, mlp_W2, mlp_b2, mlp_W3, mlp_b3):
    src = np.asarray(edge_index[0], np.int64)
    dst = np.asarray(edge_index[1], np.int64)
    # self loops for every real node (weight nrm[d]^2 folds in)
    src_all = np.concatenate([src, np.arange(N, dtype=np.int64)])
    dst_all = np.concatenate([dst, np.arange(N, dtype=np.int64)])
    order = np.argsort(dst_all, kind="stable")
    s_sorted = src_all[order]
    d_sorted = dst_all[order]

    deg = np.bincount(dst_all, minlength=NPAD).astype(np.float64)  # incl self

    nblk = NCORE * BPC
    starts = np.searchsorted(d_sorted, np.arange(nblk) * P)
    ends = np.searchsorted(d_sorted, (np.arange(nblk) + 1) * P)

    # dense adjacency per dst block, chunk-major in CHUNK_ORDER.
    # The 10 chunks OWNED by the dst core are split out into A_local (kept
    # resident in SBUF, consumed from hws_sb before the AllGather lands) and
    # zeroed in the streamed A.
    A_blocks = {}
    A_local = {}
    for g in range(nblk):
        c, nb = divmod(g, BPC)
        e_s = s_sorted[starts[g]:ends[g]]
        e_d = d_sorted[starts[g]:ends[g]] - g * P
        A = np.zeros((NPAD, P), np.float32)
        np.add.at(A, (e_s, e_d), 1.0)
        A = A.reshape(NCHUNK, P, P)
        own = A[c * BPC:(c + 1) * BPC].copy()             # [10, P, P]
        A[c * BPC:(c + 1) * BPC] = 0.0
        A = A[CHUNK_ORDER]                                # reorder chunks
        # fp8 e4m3: edge multiplicities (<= 3 incl. self loop) are exact,
        # and fp8 x fp8 DoubleRow matmul runs at 2x bf16 throughput.
        A_blocks[(c, nb)] = np.ascontiguousarray(
            A.transpose(1, 0, 2).reshape(P, NCHUNK * P)
        ).astype(ml_dtypes.float8_e4m3)
        A_local[(c, nb)] = np.ascontiguousarray(
            own.transpose(1, 0, 2).reshape(P, BPC * P)
        ).astype(ml_dtypes.float8_e4m3)

    # graph pool one-hot [node, graph] (bf16: values 0/1 exact)
    bids = np.asarray(batch_ids, np.int64)
    psel_full = np.zeros((NPAD, G), np.float32)
    psel_full[np.arange(N), bids] = 1.0
    cnt = np.bincount(bids, minlength=G).astype(np.float64)
    rcnt = (1.0 / np.maximum(cnt, 1.0)).astype(np.float32)[None, :]

    x_np = np.zeros((NPAD, NF), np.float32)
    x_np[:N] = np.asarray(x, np.float64)

    Wf = np.asarray(W, np.float32)
    W_lhsT = Wf.reshape(L, 2, P, H).transpose(2, 0, 1, 3).reshape(P, L * 2 * H)
    gm = np.asarray(gamma, np.float32)
    bt = np.asarray(beta, np.float32)
    gb = np.concatenate([gm.reshape(-1), bt.reshape(-1)])[None, :]
    embf = np.asarray(emb, np.float32)
    emb0 = np.ascontiguousarray(embf[:, 0, :])
    emb1 = np.ascontiguousarray(embf[:, 1, :])
    w1 = np.asarray(mlp_W1, np.float32).reshape(2, P, P).transpose(1, 0, 2).reshape(P, 2 * P)
    w2 = np.asarray(mlp_W2, np.float32)
    w3 = np.asarray(mlp_W3, np.float32)
    b1 = np.asarray(mlp_b1, np.float32).reshape(P, 1)
    b2 = np.asarray(mlp_b2, np.float32).reshape(64, 1)
    b3 = np.asarray(mlp_b3, np.float32).reshape(1, 1)

    in_maps = []
    for c in range(NCORE):
        lo, hi = c * NPC, (c + 1) * NPC
        # fifth-major A tiles: tile s holds ALL 10 dst blocks' columns for
        # gather piece s (16 chunks each), block-major inside.
        Ab = np.stack([A_blocks[(c, nb)] for nb in range(BPC)], axis=1)
        Ac = Ab.reshape(P, BPC, NSEG, CPS * P).transpose(0, 2, 1, 3)
        Ac = np.ascontiguousarray(Ac).reshape(P, BPC * NCHUNK * P)

        degc = deg[lo:hi].reshape(BPC, P).T
        maskc = (degc > 0).astype(np.float32)
        degc = np.maximum(degc, 1.0).astype(np.float32)

        pselc = psel_full[lo:hi].reshape(BPC, P, G)
        pselc = np.ascontiguousarray(pselc.transpose(1, 0, 2)).reshape(P, BPC * G)

        Aloc = np.concatenate([A_local[(c, nb)] for nb in range(BPC)], axis=1)
        in_maps.append(dict(
            A=Ac, Aloc=Aloc, xT=np.ascontiguousarray(x_np[lo:hi].T),
            deg=degc, mask=maskc, psel=pselc.astype(bfnp),
            W=W_lhsT.astype(bfnp), gb=gb, emb0=emb0, emb1=emb1,
            w1=w1, w2=w2, w3=w3, b1=b1, b2=b2, b3=b3, rcnt=rcnt,
        ))
    return in_maps


# --------------------------------------------------------------------------
# device program
# --------------------------------------------------------------------------

def _build():
    nc = bacc.Bacc(None, target_bir_lowering=False)

    d_A = nc.dram_tensor("A", [P, BPC * NCHUNK * P], f8, kind="ExternalInput")
    d_Aloc = nc.dram_tensor("Aloc", [P, BPC * BPC * P], f8, kind="ExternalInput")
    d_xT = nc.dram_tensor("xT", [NF, NPC], f32, kind="ExternalInput")
    d_deg = nc.dram_tensor("deg", [P, BPC], f32, kind="ExternalInput")
    d_mask = nc.dram_tensor("mask", [P, BPC], f32, kind="ExternalInput")
    d_psel = nc.dram_tensor("psel", [P, BPC * G], bf16, kind="ExternalInput")
    d_W = nc.dram_tensor("W", [P, L * 2 * H], bf16, kind="ExternalInput")
    d_gb = nc.dram_tensor("gb", [1, 2 * L * H], f32, kind="ExternalInput")
    d_emb0 = nc.dram_tensor("emb0", [NF, H], f32, kind="ExternalInput")
    d_emb1 = nc.dram_tensor("emb1", [NF, H], f32, kind="ExternalInput")
    d_w1 = nc.dram_tensor("w1", [P, 2 * P], f32, kind="ExternalInput")
    d_w2 = nc.dram_tensor("w2", [P, 64], f32, kind="ExternalInput")
    d_w3 = nc.dram_tensor("w3", [64, 1], f32, kind="ExternalInput")
    d_b1 = nc.dram_tensor("b1", [P, 1], f32, kind="ExternalInput")
    d_b2 = nc.dram_tensor("b2", [64, 1], f32, kind="ExternalInput")
    d_b3 = nc.dram_tensor("b3", [1, 1], f32, kind="ExternalInput")
    d_rcnt = nc.dram_tensor("rcnt", [1, G], f32, kind="ExternalInput")
    d_out = nc.dram_tensor("out", [1, G], f32, kind="ExternalOutput")

    rg = [list(range(NCORE))]
    SW = BPS * H         # gather-piece payload width per partition (512 cols)

    with tile.TileContext(nc) as tc, ExitStack() as ctx:
        pers = ctx.enter_context(tc.tile_pool(name="pers", bufs=1))
        psA = ctx.enter_context(tc.tile_pool(name="psA", bufs=4, space="PSUM"))
        psB = ctx.enter_context(tc.tile_pool(name="psB", bufs=2, space="PSUM"))
        apool = ctx.enter_context(tc.tile_pool(name="apool", bufs=4))
        work = ctx.enter_context(tc.tile_pool(name="work", bufs=2))
        stream = ctx.enter_context(tc.tile_pool(name="stream", bufs=2))
        dram = ctx.enter_context(tc.tile_pool(name="dram", bufs=2, space="DRAM"))

        # ---- persistent SBUF state -------------------------------------
        deg_sb = pers.tile([P, BPC], f32, tag="deg")
        mask_sb = pers.tile([P, BPC], f32, tag="mask")
        psel_sb = pers.tile([P, BPC * G], bf16, tag="psel")
        W_sb = pers.tile([P, L * 2 * H], bf16, tag="W")
        gb_sb = pers.tile([1, 2 * L * H], f32, tag="gb")
        emb0_sb = pers.tile([NF, H], f32, tag="emb0")
        emb1_sb = pers.tile([NF, H], f32, tag="emb1")
        w1_sb = pers.tile([P, 2 * P], f32, tag="w1")
        w2_sb = pers.tile([P, 64], f32, tag="w2")
        w3_sb = pers.tile([64, 1], f32, tag="w3")
        b1_sb = pers.tile([P, 1], f32, tag="b1")
        b2_sb = pers.tile([64, 1], f32, tag="b2")
        b3_sb = pers.tile([1, 1], f32, tag="b3")

        tab_sb = pers.tile([P, NCHUNK * H], f8, tag="tab")
        hT_sb = pers.tile([P, BPC * 2 * P], f32, tag="hT")
        hTb_sb = pers.tile([P, BPC * 2 * P], bf16, tag="hTb")
        hws_sb = pers.tile([P, BPC * H], f8, tag="hws")
        t_all = pers.tile([P, BPC * H], f32, tag="t_all")
        tT_sb = pers.tile([P, BPC * 2 * P], f32, tag="tT")
        nrm_sb = pers.tile([P, BPC], f32, tag="nrm")
        nrm32_sb = pers.tile([P, BPC], f32, tag="nrm32")
        nrm32x_sb = pers.tile([P, BPC], f32, tag="nrm32x")
        acc_sq = pers.tile([P, 2 * H], f32, tag="acc_sq")
        D_sb = pers.tile([NF, H], f32, tag="D")
        base_col = pers.tile([P, 2], f32, tag="base_col")
        bncol = pers.tile([P, 4], f32, tag="bncol")
        g_acc = pers.tile([P, 2 * G], f32, tag="g_acc")
        ident_bf = pers.tile([P, P], bf16, tag="ident")
        ident_f = pers.tile([P, P], f32, tag="identf")
        ones9 = pers.tile([NF, 1], f32, tag="ones9")
        ones1 = pers.tile([1, P], f32, tag="ones1")
        ones128 = pers.tile([P, 1], f32, tag="ones128")
        stv = pers.tile([1, 2 * H], f32, tag="stv")
        rcnt_sb = pers.tile([1, G], f32, tag="rcnt")
        aloc_sb = pers.tile([P, BPC * BPC * P], f8, tag="aloc")
        scal = pers.tile([1, 8 * H], f32, tag="scal")

        # ---- DRAM bounce buffers ---------------------------------------
        # AllGather pieces: ag_in[s][p, :] = hws rows for blocks {2s,2s+1}
        # (512B fp8 contiguous run per partition; ag_out row c*128+p holds
        # core c's piece-run for partition p). Collective outputs are Shared
        # scratchpad (single-writer: one output tile per collective).
        ag_ins = [dram.tile([P, SW], f8, tag=f"ag_in{s}", name=f"ag_in{s}")
                  for s in range(NSEG)]
        ag_outs = [
            [dram.tile([NCORE * P, SW], f8, tag=f"ag_out{s}_{l}", bufs=1,
                       name=f"ag_out{s}_{l}", addr_space="Shared")
             for s in range(NSEG)]
            for l in range(L)
        ]
        RREP = 32            # BN stats replication rows (payload 64KB)
        ar_in = dram.tile([RREP, 2 * H], f32, tag="ar_in")
        ar_outs = [dram.tile([RREP, 2 * H], f32, tag=f"ar_out_{l}", bufs=1,
                             name=f"ar_out_{l}", addr_space="Shared")
                   for l in range(L)]
        pr_in = dram.tile([2 * P, G], f32, tag="pr_in")
        pr_out = dram.tile([2 * P, G], f32, tag="pr_out", bufs=1,
                           addr_space="Shared")
        warm_in = dram.tile([P, 1], f32, tag="warm_in")
        warm_out = dram.tile([NCORE * P, 1], f32, tag="warm_out", bufs=1,
                             addr_space="Shared")

        # warmup collective FIRST: absorbs the one-time comms setup +
        # core-arrival skew while the encoder runs. Collectives cannot read
        # IO tensors, so bounce a tiny staged input through Internal DRAM.
        nc.sync.dma_start(out=warm_in[:], in_=d_deg[:, 0:1])
        nc.gpsimd.collective_compute(
            "AllGather", OP.bypass, replica_groups=rg,
            ins=[warm_in[:]], outs=[warm_out[:]])

        # ---- input loads ------------------------------------------------
        # Small early-needed tensors go on the sync queue ahead of the
        # encoder's xT loads; bulk tensors ride the scalar/gpsimd queues so
        # they delay neither the encoder DMAs nor the first AllGather bounce.
        for t, d in [(deg_sb, d_deg), (mask_sb, d_mask), (W_sb, d_W),
                     (gb_sb, d_gb), (emb0_sb, d_emb0), (emb1_sb, d_emb1)]:
            nc.sync.dma_start(out=t[:], in_=d[:])
        nc.scalar.dma_start(out=aloc_sb[:], in_=d_Aloc[:])
        for t, d in [(psel_sb, d_psel), (w1_sb, d_w1), (w2_sb, d_w2),
                     (w3_sb, d_w3), (b1_sb, d_b1), (b2_sb, d_b2),
                     (b3_sb, d_b3), (rcnt_sb, d_rcnt)]:
            nc.gpsimd.dma_start(out=t[:], in_=d[:])

        make_identity(nc, ident_bf[:])
        make_identity(nc, ident_f[:])
        nc.vector.memset(ones9[:], 1.0)
        nc.vector.memset(ones1[:], 1.0)
        nc.vector.memset(ones128[:], 1.0)
        nc.vector.memset(g_acc[:], 0.0)

        # nrm = rsqrt(deg) * mask ; the fp8 table is stored x32 (keeps hws
        # out of fp8e4m3 subnormals); the dst-side norm absorbs the 1/32
        rdeg = work.tile([P, BPC], f32, tag="rdeg", bufs=1)
        nc.vector.reciprocal(out=rdeg[:], in_=deg_sb[:])
        nc.scalar.activation(out=rdeg[:], in_=rdeg[:], func=FT.Sqrt)
        nc.vector.tensor_tensor(out=nrm_sb[:], in0=rdeg[:], in1=mask_sb[:], op=OP.mult)
        nc.vector.tensor_scalar_mul(nrm32_sb[:], nrm_sb[:], 1.0 / TSCALE)
        nc.vector.tensor_scalar_mul(nrm32x_sb[:], nrm_sb[:], TSCALE)

        # encoder prep: D = emb1 - emb0 ; base columns b_k = emb0_half_k^T @ 1
        nc.vector.tensor_tensor(out=D_sb[:], in0=emb1_sb[:], in1=emb0_sb[:], op=OP.subtract)
        for k in range(2):
            ps_b = psB.tile([P, 1], f32, tag="vec")
            nc.tensor.matmul(out=ps_b[:], lhsT=emb0_sb[:, k * P:(k + 1) * P],
                             rhs=ones9[:], start=True, stop=True)
            nc.vector.tensor_copy(out=base_col[:, k:k + 1], in_=ps_b[:])

        def hT(nb, k):
            return hT_sb[:, (nb * 2 + k) * P:(nb * 2 + k + 1) * P]

        def hTb(nb, k):
            return hTb_sb[:, (nb * 2 + k) * P:(nb * 2 + k + 1) * P]

        def tT(nb, k):
            return tT_sb[:, (nb * 2 + k) * P:(nb * 2 + k + 1) * P]

        def emit_gemm(l, nb):
            """hws[nb] = (h @ W[l]) * nrm * 32, fp8. lhsT is h^T directly."""
            ps_g = psA.tile([P, H], f32, tag="mm")
            for k in range(2):
                nc.tensor.matmul(
                    out=ps_g[:], lhsT=hTb(nb, k),
                    rhs=W_sb[:, (l * 2 + k) * H:(l * 2 + k + 1) * H],
                    start=(k == 0), stop=(k == 1))
            nc.vector.tensor_scalar_mul(hws_sb[:, nb * H:(nb + 1) * H],
                                        ps_g[:], nrm32x_sb[:, nb:nb + 1])

        def emit_ag_piece(l, s):
            nc.sync.dma_start(out=ag_ins[s][:],
                              in_=hws_sb[:, s * SW:(s + 1) * SW])
            nc.gpsimd.collective_compute(
                "AllGather", OP.bypass, replica_groups=rg,
                ins=[ag_ins[s][:]], outs=[ag_outs[l][s][:]])

        a_fifo = []

        def a_prefetch(s):
            # one fifth-tile: all 10 dst blocks x 16 chunks of gather piece s
            # (2.6MB). On the scalar queue so the ag_in bounces on sync are
            # never stuck behind a bulk transfer.
            a_t = apool.tile([P, BPC * CPS * P], f8, tag="A")
            nc.scalar.dma_start(
                out=a_t[:], in_=d_A[:, s * BPC * CPS * P:(s + 1) * BPC * CPS * P])
            a_fifo.append(a_t)

        # Aggregation runs in NSEG segments; segment s consumes gather piece
        # s for all 10 dst blocks, in chunk PAIRS via fp8 DoubleRow matmuls
        # (contraction 256/instruction at 2 fp8 rows/cycle).
        def emit_seg_chain(a_t, nb, s):
            ps_t = psA.tile([P, H], f32, tag="mm")
            for t in range(CPS // 2):
                lhsT3 = a_t[:, (nb * CPS + 2 * t) * P:
                            (nb * CPS + 2 * t + 2) * P].rearrange(
                    "p (j m) -> p j m", j=2)
                rhs3 = tab_sb[:, (s * CPS + 2 * t) * H:
                              (s * CPS + 2 * t + 2) * H].rearrange(
                    "p (j n) -> p j n", j=2)
                nc.tensor.matmul(
                    out=ps_t[:], lhsT=lhsT3, rhs=rhs3,
                    start=(t == 0), stop=(t == CPS // 2 - 1), perf_mode=DR)
            tsl = t_all[:, nb * H:(nb + 1) * H]
            nc.vector.tensor_tensor(out=tsl, in0=tsl, in1=ps_t[:], op=OP.add)
            if s == NSEG - 1:
                # t = (nrm/32)*sum; BN stats accumulate on the idle GpSimd
                # engine so the DVE queue never gates the stats matmul
                nc.vector.tensor_scalar_mul(tsl, tsl, nrm32_sb[:, nb:nb + 1])
                sq = work.tile([P, H], f32, tag="tmp3")
                nc.gpsimd.tensor_tensor(out=sq[:], in0=tsl, in1=tsl, op=OP.mult)
                nc.gpsimd.tensor_tensor(out=acc_sq[:, 0:H], in0=acc_sq[:, 0:H],
                                        in1=tsl, op=OP.add)
                nc.gpsimd.tensor_tensor(out=acc_sq[:, H:2 * H],
                                        in0=acc_sq[:, H:2 * H], in1=sq[:], op=OP.add)

        # encoder: h0^T = D^T x^T + base (directly transposed)
        for nb in range(BPC):
            xT_t = stream.tile([NF, P], f32, tag="xT_t")
            nc.sync.dma_start(out=xT_t[:], in_=d_xT[:, nb * P:(nb + 1) * P])
            for k in range(2):
                ps_h = psA.tile([P, H], f32, tag="mm")
                nc.tensor.matmul(out=ps_h[:, 0:P], lhsT=D_sb[:, k * P:(k + 1) * P],
                                 rhs=xT_t[:], start=True, stop=True)
                nc.vector.tensor_scalar_add(hT(nb, k), ps_h[:, 0:P],
                                            base_col[:, k:k + 1])
                nc.vector.tensor_copy(out=hTb(nb, k), in_=hT(nb, k))
            emit_gemm(0, nb)
            if nb % 2 == 1:
                emit_ag_piece(0, nb // 2)
                if nb == 1:
                    a_prefetch(0)
                elif nb == 5:
                    a_prefetch(1)
                elif nb == 9:
                    a_prefetch(2)

        # ---- layers -----------------------------------------------------
        for l in range(L):
            # Table loads ride the gpsimd queue: each waits on its gather
            # piece, exactly the order the Comms engine completes them, so
            # nothing else ever queues behind a blocked trigger.
            for s in range(NSEG):
                nc.gpsimd.dma_start(
                    out=tab_sb[:, s * CPS * H:(s + 1) * CPS * H].rearrange(
                        "p (c w) -> p c w", c=NCORE),
                    in_=ag_outs[l][s][:].rearrange("(c p) w -> p c w", p=P))
            nc.gpsimd.memset(acc_sq[:], 0.0)
            # local mini-chains: the core's own 10 chunks consumed straight
            # from hws_sb while the AllGather pieces are still in flight
            for nb in range(BPC):
                ps_l = psA.tile([P, H], f32, tag="mm")
                for j in range(BPC // 2):
                    lhsT3 = aloc_sb[:, (nb * BPC + 2 * j) * P:
                                    (nb * BPC + 2 * j + 2) * P].rearrange(
                        "p (k m) -> p k m", k=2)
                    rhs3 = hws_sb[:, 2 * j * H:(2 * j + 2) * H].rearrange(
                        "p (k n) -> p k n", k=2)
                    nc.tensor.matmul(
                        out=ps_l[:], lhsT=lhsT3, rhs=rhs3,
                        start=(j == 0), stop=(j == BPC // 2 - 1), perf_mode=DR)
                nc.vector.tensor_copy(out=t_all[:, nb * H:(nb + 1) * H], in_=ps_l[:])
            for s in range(NSEG):
                if s < 2:
                    a_prefetch(s + 3)
                a_t = a_fifo.pop(0)
                for nb in range(BPC):
                    emit_seg_chain(a_t, nb, s)

            # stats: one cross-partition reduce, 32-row replicate, AllReduce
            ps_s = psB.tile([1, 2 * H], f32, tag="vec")
            nc.tensor.matmul(out=ps_s[:], lhsT=ones128[:], rhs=acc_sq[:],
                             start=True, stop=True)
            st_sb = scal[:, 6 * H:8 * H]
            nc.vector.tensor_copy(out=st_sb, in_=ps_s[:])
            st_rep = work.tile([RREP, 2 * H], f32, tag="strep", bufs=1)
            ps_r2 = psB.tile([RREP, 2 * H], f32, tag="vec")
            nc.tensor.matmul(out=ps_r2[:], lhsT=ones1[:, 0:RREP], rhs=st_sb,
                             start=True, stop=True)
            nc.vector.tensor_copy(out=st_rep[:], in_=ps_r2[:])
            nc.sync.dma_start(out=ar_in[:], in_=st_rep[:])
            nc.gpsimd.collective_compute(
                "AllReduce", OP.add, replica_groups=rg,
                ins=[ar_in[:]], outs=[ar_outs[l][:]])
            nc.sync.dma_start(out=stv[:], in_=ar_outs[l][0:1, :])

            # transpose t into tT while the AllReduce is in flight (TensorE
            # is otherwise idle in this window)
            for nb in range(BPC):
                for k in range(2):
                    ps_t2 = psB.tile([P, P], f32, tag="pst")
                    nc.tensor.transpose(
                        out=ps_t2[:], in_=t_all[:, nb * H + k * P:nb * H + (k + 1) * P],
                        identity=ident_f[:])
                    nc.vector.tensor_copy(out=tT(nb, k), in_=ps_t2[:])

            # BN scalar math on [1,256] rows: a = gamma*istd, c = beta - mu*a
            mu = scal[:, H:2 * H]
            var = scal[:, 2 * H:3 * H]
            msq = scal[:, 5 * H:6 * H]
            nc.vector.tensor_scalar_mul(mu, stv[:, 0:H], 1.0 / N)
            nc.vector.tensor_scalar_mul(var, stv[:, H:2 * H], 1.0 / N)
            nc.vector.tensor_tensor(out=msq, in0=mu, in1=mu, op=OP.mult)
            nc.vector.tensor_tensor(out=var, in0=var, in1=msq, op=OP.subtract)
            nc.vector.tensor_scalar_add(var, var, BN_EPS)
            nc.vector.reciprocal_approx_fast(out=var, in_=var)
            nc.scalar.activation(out=var, in_=var, func=FT.Sqrt)  # istd
            av = scal[:, 3 * H:4 * H]
            cv = scal[:, 4 * H:5 * H]
            nc.vector.tensor_tensor(out=av, in0=var,
                                    in1=gb_sb[:, l * H:(l + 1) * H], op=OP.mult)
            nc.vector.tensor_tensor(out=msq, in0=mu, in1=av, op=OP.mult)
            nc.vector.tensor_tensor(out=cv, in0=gb_sb[:, (L + l) * H:(L + l + 1) * H],
                                    in1=msq, op=OP.subtract)
            # column-ize a||c: 4 tiny matmuls [1,128]^T @ [1,1] -> [128,1]
            ps_col = psB.tile([P, 4], f32, tag="vec")
            for j in range(4):
                nc.tensor.matmul(out=ps_col[:, j:j + 1],
                                 lhsT=scal[:, 3 * H + j * P:3 * H + (j + 1) * P],
                                 rhs=ones1[:, 0:1], start=True, stop=True,
                                 skip_group_check=True)
            nc.vector.tensor_copy(out=bncol[:], in_=ps_col[:])

            # apply: h^T += relu(t^T * a + c) per (block, half); fused DVE
            # per-partition scalar op + Scalar-engine relu. Immediately GEMM
            # the updated block for the next layer; post the AllGather halves
            # as soon as each half's blocks are done.
            for nb in range(BPC):
                for k in range(2):
                    u = work.tile([P, P], f32, tag="tmp")
                    nc.vector.tensor_scalar(
                        out=u[:], in0=tT(nb, k),
                        scalar1=bncol[:, k:k + 1], scalar2=bncol[:, 2 + k:3 + k],
                        op0=OP.mult, op1=OP.add)
                    r = work.tile([P, P], f32, tag="tmp2")
                    nc.scalar.activation(out=r[:], in_=u[:], func=FT.Relu)
                    nc.vector.tensor_tensor(out=hT(nb, k), in0=hT(nb, k),
                                            in1=r[:], op=OP.add)
                    nc.vector.tensor_copy(out=hTb(nb, k), in_=hT(nb, k))
                if l < L - 1:
                    emit_gemm(l + 1, nb)
                    if nb % 2 == 1:
                        emit_ag_piece(l + 1, nb // 2)
                        if nb == 1:
                            a_prefetch(0)
                        elif nb == 5:
                            a_prefetch(1)
                        elif nb == 9:
                            a_prefetch(2)
                else:
                    # last layer: transpose back to node-major, pool matmuls,
                    # accumulate in SBUF (keeps PSUM banks free for chains)
                    hb_t = work.tile([P, H], bf16, tag="hb")
                    for k in range(2):
                        ps_tr = psB.tile([P, P], bf16, tag="pst")
                        nc.tensor.transpose(out=ps_tr[:], in_=hTb(nb, k),
                                            identity=ident_bf[:])
                        nc.vector.tensor_copy(out=hb_t[:, k * P:(k + 1) * P],
                                              in_=ps_tr[:])
                    pssl = psel_sb[:, nb * G:(nb + 1) * G]
                    for k in range(2):
                        ps_p = psB.tile([P, G], f32, tag="vec")
                        nc.tensor.matmul(out=ps_p[:], lhsT=hb_t[:, k * P:(k + 1) * P],
                                         rhs=pssl, start=True, stop=True)
                        nc.vector.tensor_tensor(
                            out=g_acc[:, k * G:(k + 1) * G],
                            in0=g_acc[:, k * G:(k + 1) * G], in1=ps_p[:], op=OP.add)

        # ---- pooling readout --------------------------------------------
        nc.sync.dma_start(out=pr_in[0:P, :], in_=g_acc[:, 0:G])
        nc.sync.dma_start(out=pr_in[P:2 * P, :], in_=g_acc[:, G:2 * G])
        nc.gpsimd.collective_compute(
            "AllReduce", OP.add, replica_groups=rg,
            ins=[pr_in[:]], outs=[pr_out[:]])
        g0 = work.tile([P, G], f32, tag="g0", bufs=1)
        g1 = work.tile([P, G], f32, tag="g1", bufs=1)
        nc.sync.dma_start(out=g0[:], in_=pr_out[0:P, :])
        nc.sync.dma_start(out=g1[:], in_=pr_out[P:2 * P, :])
        ps_r = psB.tile([P, G], f32, tag="vec")
        nc.tensor.matmul(out=ps_r[:], lhsT=ones1[:], rhs=rcnt_sb[:], start=True, stop=True)
        rc_rep = work.tile([P, G], f32, tag="rc_rep", bufs=1)
        nc.vector.tensor_copy(out=rc_rep[:], in_=ps_r[:])
        nc.vector.tensor_tensor(out=g0[:], in0=g0[:], in1=rc_rep[:], op=OP.mult)
        nc.vector.tensor_tensor(out=g1[:], in0=g1[:], in1=rc_rep[:], op=OP.mult)

        # MLP head (transposed: weights are lhsT, graphs along free dim)
        ps1 = psB.tile([P, G], f32, tag="vec")
        nc.tensor.matmul(out=ps1[:], lhsT=w1_sb[:, 0:P], rhs=g0[:], start=True, stop=False)
        nc.tensor.matmul(out=ps1[:], lhsT=w1_sb[:, P:2 * P], rhs=g1[:], start=False, stop=True)
        y1 = work.tile([P, G], f32, tag="y1", bufs=1)
        nc.scalar.activation(out=y1[:], in_=ps1[:], func=FT.Relu, bias=b1_sb[:, 0:1])
        ps2 = psB.tile([64, G], f32, tag="vec")
        nc.tensor.matmul(out=ps2[:], lhsT=w2_sb[:], rhs=y1[:], start=True, stop=True)
        y2 = work.tile([64, G], f32, tag="y2", bufs=1)
        nc.scalar.activation(out=y2[:], in_=ps2[:], func=FT.Relu, bias=b2_sb[:, 0:1])
        ps3 = psB.tile([1, G], f32, tag="vec")
        nc.tensor.matmul(out=ps3[:], lhsT=w3_sb[:], rhs=y2[:], start=True, stop=True)
        y3 = work.tile([1, G], f32, tag="y3", bufs=1)
        nc.vector.tensor_scalar_add(y3[:], ps3[:], b3_sb[0:1, 0:1])
        nc.sync.dma_start(out=d_out[:], in_=y3[:])

    nc.compile()
    return nc


# --------------------------------------------------------------------------
# entry point
# --------------------------------------------------------------------------

def kernel(x, edge_index, batch_ids, emb, W, b, gamma, beta,
           mlp_W1, mlp_b1, mlp_W2, mlp_b2, mlp_W3, mlp_b3,
           _trace=False, _trace_kwargs=None):
    # NB: reference BN subtracts the per-channel mean, so the additive bias b
    # cancels exactly and is not needed by the device program.
    in_maps = _preprocess(x, edge_index, batch_ids, emb, W, gamma, beta,
                          mlp_W1, mlp_b1, mlp_W2, mlp_b2, mlp_W3, mlp_b3)
    if "nc" not in _compiled:
        _compiled["nc"] = _build()
    nc = _compiled["nc"]
    kw = {}
    if _trace:
        kw = dict(trace=True, **(_trace_kwargs or {}))
    res = run_bass_kernel_spmd(nc, in_maps, core_ids=list(range(NCORE)), **kw)
    out = np.asarray(res.results[0]["out"], np.float32).reshape(G, 1)
    kernel._last_results = res
    return out


# revision 59
# speedup vs baseline: 1.0121x; 1.0121x over previous
"""Trainium2 Bass kernel for HIVNet GCN message passing (8-core SPMD).

v8 strategy (baseline 2.29ms -> v6 pure-dense 826us -> v7 DoubleRow 638us):
  - Pad N=10000 nodes to 10240 = 80 chunks x 128; core c owns 10 dst-blocks
    (global chunks c*10..c*10+9).
  - Per layer: hws = (h @ W[l])*nrm*32 on the owned shard, cast fp8e4m3,
    AllGather the partition-major table in two halves; aggregation is pure
    dense one-hot adjacency on TensorE using fp8 DoubleRow matmuls
    (both operands fp8, contraction 256/instruction, 2x bf16 throughput).
    The x32 table scale keeps hws out of fp8 subnormals; the dst-side norm
    carries the 1/32.
  - h lives TRANSPOSED (h^T: H on partitions, nodes on free dim):
      * the next-layer GEMM consumes h^T directly as lhsT (no transposes),
      * BN apply is one fused per-partition tensor_scalar (t^T*a + c) + relu,
      * t^T transposes run inside the BN AllReduce window (Tensor idle),
      * BN scale/shift column-ized via 4 tiny matmuls (no 128-row bcast).
  - BN stats: fused sum||sumsq reduce, 32-row replicate, Shared-output
    AllReduce; warmup AllGather at t=0 absorbs comms cold-start skew.
  - Readout: transpose h back per block (last layer only), one-hot pool
    matmuls accumulated in SBUF, 257-row AllReduce, redundant 3-layer MLP.
"""

import os
import sys

sys.path.insert(0, "/opt/trn_rl_repo")

from contextlib import ExitStack

import numpy as np
import ml_dtypes

from concourse import bass, mybir, bacc, tile, library_config
from concourse.bass_utils import run_bass_kernel_spmd
from concourse.masks import make_identity

NCORE = 8
P = 128
H = 256
L = 4
NF = 9
G = 256
N = 10000
BPC = 10                # dst blocks per core
NPC = BPC * P           # 1280 nodes per core
NPAD = NCORE * NPC      # 10240
NCHUNK = NPAD // P      # 80 src chunks
HB = BPC // 2           # blocks per AllGather half
BN_EPS = 1e-5
TSCALE = 32.0           # fp8 table scale

f32 = mybir.dt.float32
bf16 = mybir.dt.bfloat16
f8 = mybir.dt.float8e4
bfnp = ml_dtypes.bfloat16

FT = mybir.ActivationFunctionType
OP = mybir.AluOpType
DR = mybir.MatmulPerfMode.DoubleRow

_compiled = {}

NSEG = 5                # AllGather pieces per layer (2 blocks each)
BPS = BPC // NSEG       # blocks per gather piece
CPS = NCORE * BPS       # chunks per gather piece (16)

# chunk consumption order: fifth-major (blocks {2s,2s+1} of every core form
# gather piece s), so dense-chain segment s can start as soon as piece s
# lands; within a piece, core-major ascending = the gathered tab layout.
CHUNK_ORDER = [g for s in range(NSEG) for g in range(NCHUNK)
               if g % BPC in (2 * s, 2 * s + 1)]


# --------------------------------------------------------------------------
# host-side structural preprocessing
# --------------------------------------------------------------------------

def _preprocess(x, edge_index, batch_ids, emb, W, gamma, beta,
                mlp_W1, mlp_b1, mlp_W2, mlp_b2, mlp_W3, mlp_b3):
    src = np.asarray(edge_index[0], np.int64)
    dst = np.asarray(edge_index[1], np.int64)
    # self loops for every real node (weight nrm[d]^2 folds in)
    src_all = np.concatenate([src, np.arange(N, dtype=np.int64)])
    dst_all = np.concatenate([dst, np.arange(N, dtype=np.int64)])
    order = np.argsort(dst_all, kind="stable")
    s_sorted = src_all[order]
    d_sorted = dst_all[order]

    deg = np.bincount(dst_all, minlength=NPAD).astype(np.float64)  # incl self

    nblk = NCORE * BPC
    starts = np.searchsorted(d_sorted, np.arange(nblk) * P)
    ends = np.searchsorted(d_sorted, (np.arange(nblk) + 1) * P)

    # dense adjacency per dst block, chunk-major in CHUNK_ORDER.
    # The 10 chunks OWNED by the dst core are split out into A_local (kept
    # resident in SBUF, consumed from hws_sb before the AllGather lands) and
    # zeroed in the streamed A.
    A_blocks = {}
    A_local = {}
    for g in range(nblk):
        c, nb = divmod(g, BPC)
        e_s = s_sorted[starts[g]:ends[g]]
        e_d = d_sorted[starts[g]:ends[g]] - g * P
        A = np.zeros((NPAD, P), np.float32)
        np.add.at(A, (e_s, e_d), 1.0)
        A = A.reshape(NCHUNK, P, P)
        own = A[c * BPC:(c + 1) * BPC].copy()             # [10, P, P]
        A[c * BPC:(c + 1) * BPC] = 0.0
        A = A[CHUNK_ORDER]                                # reorder chunks
        # fp8 e4m3: edge multiplicities (<= 3 incl. self loop) are exact,
        # and fp8 x fp8 DoubleRow matmul runs at 2x bf16 throughput.
        A_blocks[(c, nb)] = np.ascontiguousarray(
            A.transpose(1, 0, 2).reshape(P, NCHUNK * P)
        ).astype(ml_dtypes.float8_e4m3)
        A_local[(c, nb)] = np.ascontiguousarray(
            own.transpose(1, 0, 2).reshape(P, BPC * P)
        ).astype(ml_dtypes.float8_e4m3)

    # graph pool one-hot [node, graph] (bf16: values 0/1 exact)
    bids = np.asarray(batch_ids, np.int64)
    psel_full = np.zeros((NPAD, G), np.float32)
    psel_full[np.arange(N), bids] = 1.0
    cnt = np.bincount(bids, minlength=G).astype(np.float64)
    rcnt = (1.0 / np.maximum(cnt, 1.0)).astype(np.float32)[None, :]

    x_np = np.zeros((NPAD, NF), np.float32)
    x_np[:N] = np.asarray(x, np.float64)

    Wf = np.asarray(W, np.float32)
    W_lhsT = Wf.reshape(L, 2, P, H).transpose(2, 0, 1, 3).reshape(P, L * 2 * H)
    gm = np.asarray(gamma, np.float32)
    bt = np.asarray(beta, np.float32)
    gb = np.concatenate([gm.reshape(-1), bt.reshape(-1)])[None, :]
    embf = np.asarray(emb, np.float32)
    emb0 = np.ascontiguousarray(embf[:, 0, :])
    emb1 = np.ascontiguousarray(embf[:, 1, :])
    w1 = np.asarray(mlp_W1, np.float32).reshape(2, P, P).transpose(1, 0, 2).reshape(P, 2 * P)
    w2 = np.asarray(mlp_W2, np.float32)
    w3 = np.asarray(mlp_W3, np.float32)
    b1 = np.asarray(mlp_b1, np.float32).reshape(P, 1)
    b2 = np.asarray(mlp_b2, np.float32).reshape(64, 1)
    b3 = np.asarray(mlp_b3, np.float32).reshape(1, 1)

    in_maps = []
    for c in range(NCORE):
        lo, hi = c * NPC, (c + 1) * NPC
        # fifth-major A tiles: tile s holds ALL 10 dst blocks' columns for
        # gather piece s (16 chunks each), block-major inside.
        Ab = np.stack([A_blocks[(c, nb)] for nb in range(BPC)], axis=1)
        Ac = Ab.reshape(P, BPC, NSEG, CPS * P).transpose(0, 2, 1, 3)
        Ac = np.ascontiguousarray(Ac).reshape(P, BPC * NCHUNK * P)

        degc = deg[lo:hi].reshape(BPC, P).T
        maskc = (degc > 0).astype(np.float32)
        degc = np.maximum(degc, 1.0).astype(np.float32)

        pselc = psel_full[lo:hi].reshape(BPC, P, G)
        pselc = np.ascontiguousarray(pselc.transpose(1, 0, 2)).reshape(P, BPC * G)

        Aloc = np.concatenate([A_local[(c, nb)] for nb in range(BPC)], axis=1)
        in_maps.append(dict(
            A=Ac, Aloc=Aloc, xT=np.ascontiguousarray(x_np[lo:hi].T),
            deg=degc, mask=maskc, psel=pselc.astype(bfnp),
            W=W_lhsT.astype(bfnp), gb=gb, emb0=emb0, emb1=emb1,
            w1=w1, w2=w2, w3=w3, b1=b1, b2=b2, b3=b3, rcnt=rcnt,
        ))
    return in_maps


# --------------------------------------------------------------------------
# device program
# --------------------------------------------------------------------------

def _build():
    nc = bacc.Bacc(None, target_bir_lowering=False)

    d_A = nc.dram_tensor("A", [P, BPC * NCHUNK * P], f8, kind="ExternalInput")
    d_Aloc = nc.dram_tensor("Aloc", [P, BPC * BPC * P], f8, kind="ExternalInput")
    d_xT = nc.dram_tensor("xT", [NF, NPC], f32, kind="ExternalInput")
    d_deg = nc.dram_tensor("deg", [P, BPC], f32, kind="ExternalInput")
    d_mask = nc.dram_tensor("mask", [P, BPC], f32, kind="ExternalInput")
    d_psel = nc.dram_tensor("psel", [P, BPC * G], bf16, kind="ExternalInput")
    d_W = nc.dram_tensor("W", [P, L * 2 * H], bf16, kind="ExternalInput")
    d_gb = nc.dram_tensor("gb", [1, 2 * L * H], f32, kind="ExternalInput")
    d_emb0 = nc.dram_tensor("emb0", [NF, H], f32, kind="ExternalInput")
    d_emb1 = nc.dram_tensor("emb1", [NF, H], f32, kind="ExternalInput")
    d_w1 = nc.dram_tensor("w1", [P, 2 * P], f32, kind="ExternalInput")
    d_w2 = nc.dram_tensor("w2", [P, 64], f32, kind="ExternalInput")
    d_w3 = nc.dram_tensor("w3", [64, 1], f32, kind="ExternalInput")
    d_b1 = nc.dram_tensor("b1", [P, 1], f32, kind="ExternalInput")
    d_b2 = nc.dram_tensor("b2", [64, 1], f32, kind="ExternalInput")
    d_b3 = nc.dram_tensor("b3", [1, 1], f32, kind="ExternalInput")
    d_rcnt = nc.dram_tensor("rcnt", [1, G], f32, kind="ExternalInput")
    d_out = nc.dram_tensor("out", [1, G], f32, kind="ExternalOutput")

    rg = [list(range(NCORE))]
    SW = BPS * H         # gather-piece payload width per partition (512 cols)

    with tile.TileContext(nc) as tc, ExitStack() as ctx:
        pers = ctx.enter_context(tc.tile_pool(name="pers", bufs=1))
        psA = ctx.enter_context(tc.tile_pool(name="psA", bufs=4, space="PSUM"))
        psB = ctx.enter_context(tc.tile_pool(name="psB", bufs=2, space="PSUM"))
        apool = ctx.enter_context(tc.tile_pool(name="apool", bufs=4))
        work = ctx.enter_context(tc.tile_pool(name="work", bufs=2))
        stream = ctx.enter_context(tc.tile_pool(name="stream", bufs=2))
        dram = ctx.enter_context(tc.tile_pool(name="dram", bufs=2, space="DRAM"))

        # ---- persistent SBUF state -------------------------------------
        deg_sb = pers.tile([P, BPC], f32, tag="deg")
        mask_sb = pers.tile([P, BPC], f32, tag="mask")
        psel_sb = pers.tile([P, BPC * G], bf16, tag="psel")
        W_sb = pers.tile([P, L * 2 * H], bf16, tag="W")
        gb_sb = pers.tile([1, 2 * L * H], f32, tag="gb")
        emb0_sb = pers.tile([NF, H], f32, tag="emb0")
        emb1_sb = pers.tile([NF, H], f32, tag="emb1")
        w1_sb = pers.tile([P, 2 * P], f32, tag="w1")
        w2_sb = pers.tile([P, 64], f32, tag="w2")
        w3_sb = pers.tile([64, 1], f32, tag="w3")
        b1_sb = pers.tile([P, 1], f32, tag="b1")
        b2_sb = pers.tile([64, 1], f32, tag="b2")
        b3_sb = pers.tile([1, 1], f32, tag="b3")

        tab_sb = pers.tile([P, NCHUNK * H], f8, tag="tab")
        hT_sb = pers.tile([P, BPC * 2 * P], f32, tag="hT")
        hTb_sb = pers.tile([P, BPC * 2 * P], bf16, tag="hTb")
        hws_sb = pers.tile([P, BPC * H], f8, tag="hws")
        t_all = pers.tile([P, BPC * H], f32, tag="t_all")
        tT_sb = pers.tile([P, BPC * 2 * P], f32, tag="tT")
        nrm_sb = pers.tile([P, BPC], f32, tag="nrm")
        nrm32_sb = pers.tile([P, BPC], f32, tag="nrm32")
        nrm32x_sb = pers.tile([P, BPC], f32, tag="nrm32x")
        acc_sq = pers.tile([P, 2 * H], f32, tag="acc_sq")
        D_sb = pers.tile([NF, H], f32, tag="D")
        base_col = pers.tile([P, 2], f32, tag="base_col")
        bncol = pers.tile([P, 4], f32, tag="bncol")
        g_acc = pers.tile([P, 2 * G], f32, tag="g_acc")
        ident_bf = pers.tile([P, P], bf16, tag="ident")
        ident_f = pers.tile([P, P], f32, tag="identf")
        ones9 = pers.tile([NF, 1], f32, tag="ones9")
        ones1 = pers.tile([1, P], f32, tag="ones1")
        ones128 = pers.tile([P, 1], f32, tag="ones128")
        stv = pers.tile([1, 2 * H], f32, tag="stv")
        rcnt_sb = pers.tile([1, G], f32, tag="rcnt")
        aloc_sb = pers.tile([P, BPC * BPC * P], f8, tag="aloc")
        scal = pers.tile([1, 8 * H], f32, tag="scal")

        # ---- DRAM bounce buffers ---------------------------------------
        # AllGather pieces: ag_in[s][p, :] = hws rows for blocks {2s,2s+1}
        # (512B fp8 contiguous run per partition; ag_out row c*128+p holds
        # core c's piece-run for partition p). Collective outputs are Shared
        # scratchpad (single-writer: one output tile per collective).
        # AG outputs stay Local: with a Shared output the collective is cheap
        # but the table load then pulls 7/8 of its bytes from remote HBM on
        # the critical path; Local keeps the transport inside the collective
        # (overlapped with compute) and the tab read fast.
        ag_ins = [dram.tile([P, SW], f8, tag=f"ag_in{s}", name=f"ag_in{s}")
                  for s in range(NSEG)]
        ag_outs = [
            [dram.tile([NCORE * P, SW], f8, tag=f"ag_out{s}_{l}", bufs=1,
                       name=f"ag_out{s}_{l}")
             for s in range(NSEG)]
            for l in range(L)
        ]
        RREP = 32            # BN stats replication rows (payload 64KB)
        ar_in = dram.tile([RREP, 2 * H], f32, tag="ar_in")
        ar_outs = [dram.tile([RREP, 2 * H], f32, tag=f"ar_out_{l}", bufs=1,
                             name=f"ar_out_{l}", addr_space="Shared")
                   for l in range(L)]
        pr_in = dram.tile([2 * P, G], f32, tag="pr_in")
        pr_out = dram.tile([2 * P, G], f32, tag="pr_out", bufs=1,
                           addr_space="Shared")


        # ---- input loads ------------------------------------------------
        # Small early-needed tensors go on the sync queue ahead of the
        # encoder's xT loads; bulk tensors ride the scalar/gpsimd queues so
        # they delay neither the encoder DMAs nor the first AllGather bounce.
        for t, d in [(deg_sb, d_deg), (mask_sb, d_mask), (W_sb, d_W),
                     (gb_sb, d_gb), (emb0_sb, d_emb0), (emb1_sb, d_emb1)]:
            nc.sync.dma_start(out=t[:], in_=d[:])
        nc.scalar.dma_start(out=aloc_sb[:], in_=d_Aloc[:])
        for t, d in [(psel_sb, d_psel), (w1_sb, d_w1), (w2_sb, d_w2),
                     (w3_sb, d_w3), (b1_sb, d_b1), (b2_sb, d_b2),
                     (b3_sb, d_b3), (rcnt_sb, d_rcnt)]:
            nc.gpsimd.dma_start(out=t[:], in_=d[:])

        make_identity(nc, ident_bf[:])
        make_identity(nc, ident_f[:])
        nc.vector.memset(ones9[:], 1.0)
        nc.vector.memset(ones1[:], 1.0)
        nc.vector.memset(ones128[:], 1.0)
        nc.vector.memset(g_acc[:], 0.0)

        # nrm = rsqrt(deg) * mask ; the fp8 table is stored x32 (keeps hws
        # out of fp8e4m3 subnormals); the dst-side norm absorbs the 1/32
        rdeg = work.tile([P, BPC], f32, tag="rdeg", bufs=1)
        nc.vector.reciprocal(out=rdeg[:], in_=deg_sb[:])
        nc.scalar.activation(out=rdeg[:], in_=rdeg[:], func=FT.Sqrt)
        nc.vector.tensor_tensor(out=nrm_sb[:], in0=rdeg[:], in1=mask_sb[:], op=OP.mult)
        nc.vector.tensor_scalar_mul(nrm32_sb[:], nrm_sb[:], 1.0 / TSCALE)
        nc.vector.tensor_scalar_mul(nrm32x_sb[:], nrm_sb[:], TSCALE)

        # encoder prep: D = emb1 - emb0 ; base columns b_k = emb0_half_k^T @ 1
        nc.vector.tensor_tensor(out=D_sb[:], in0=emb1_sb[:], in1=emb0_sb[:], op=OP.subtract)
        for k in range(2):
            ps_b = psB.tile([P, 1], f32, tag="vec")
            nc.tensor.matmul(out=ps_b[:], lhsT=emb0_sb[:, k * P:(k + 1) * P],
                             rhs=ones9[:], start=True, stop=True)
            nc.vector.tensor_copy(out=base_col[:, k:k + 1], in_=ps_b[:])

        def hT(nb, k):
            return hT_sb[:, (nb * 2 + k) * P:(nb * 2 + k + 1) * P]

        def hTb(nb, k):
            return hTb_sb[:, (nb * 2 + k) * P:(nb * 2 + k + 1) * P]

        def tT(nb, k):
            return tT_sb[:, (nb * 2 + k) * P:(nb * 2 + k + 1) * P]

        def emit_gemm(l, nb):
            """hws[nb] = (h @ W[l]) * nrm * 32, fp8. lhsT is h^T directly."""
            ps_g = psA.tile([P, H], f32, tag="mm")
            for k in range(2):
                nc.tensor.matmul(
                    out=ps_g[:], lhsT=hTb(nb, k),
                    rhs=W_sb[:, (l * 2 + k) * H:(l * 2 + k + 1) * H],
                    start=(k == 0), stop=(k == 1))
            nc.vector.tensor_scalar_mul(hws_sb[:, nb * H:(nb + 1) * H],
                                        ps_g[:], nrm32x_sb[:, nb:nb + 1])

        def emit_ag_piece(l, s):
            nc.sync.dma_start(out=ag_ins[s][:],
                              in_=hws_sb[:, s * SW:(s + 1) * SW])
            nc.gpsimd.collective_compute(
                "AllGather", OP.bypass, replica_groups=rg,
                ins=[ag_ins[s][:]], outs=[ag_outs[l][s][:]])

        a_fifo = []

        def a_prefetch(s):
            # one fifth-tile: all 10 dst blocks x 16 chunks of gather piece s
            # (2.6MB). On the scalar queue so the ag_in bounces on sync are
            # never stuck behind a bulk transfer.
            a_t = apool.tile([P, BPC * CPS * P], f8, tag="A")
            nc.scalar.dma_start(
                out=a_t[:], in_=d_A[:, s * BPC * CPS * P:(s + 1) * BPC * CPS * P])
            a_fifo.append(a_t)

        # Aggregation runs in NSEG segments; segment s consumes gather piece
        # s for all 10 dst blocks, in chunk PAIRS via fp8 DoubleRow matmuls
        # (contraction 256/instruction at 2 fp8 rows/cycle).
        def emit_seg_chain(a_t, nb, s):
            ps_t = psA.tile([P, H], f32, tag="mm")
            for t in range(CPS // 2):
                lhsT3 = a_t[:, (nb * CPS + 2 * t) * P:
                            (nb * CPS + 2 * t + 2) * P].rearrange(
                    "p (j m) -> p j m", j=2)
                rhs3 = tab_sb[:, (s * CPS + 2 * t) * H:
                              (s * CPS + 2 * t + 2) * H].rearrange(
                    "p (j n) -> p j n", j=2)
                nc.tensor.matmul(
                    out=ps_t[:], lhsT=lhsT3, rhs=rhs3,
                    start=(t == 0), stop=(t == CPS // 2 - 1), perf_mode=DR)
            tsl = t_all[:, nb * H:(nb + 1) * H]
            nc.vector.tensor_tensor(out=tsl, in0=tsl, in1=ps_t[:], op=OP.add)
            if s == NSEG - 1:
                # t = (nrm/32)*sum; BN stats accumulate on the idle GpSimd
                # engine so the DVE queue never gates the stats matmul
                nc.vector.tensor_scalar_mul(tsl, tsl, nrm32_sb[:, nb:nb + 1])
                sq = work.tile([P, H], f32, tag="tmp3")
                nc.gpsimd.tensor_tensor(out=sq[:], in0=tsl, in1=tsl, op=OP.mult)
                nc.gpsimd.tensor_tensor(out=acc_sq[:, 0:H], in0=acc_sq[:, 0:H],
                                        in1=tsl, op=OP.add)
                nc.gpsimd.tensor_tensor(out=acc_sq[:, H:2 * H],
                                        in0=acc_sq[:, H:2 * H], in1=sq[:], op=OP.add)

        # encoder: h0^T = D^T x^T + base (directly transposed)
        for nb in range(BPC):
            xT_t = stream.tile([NF, P], f32, tag="xT_t")
            nc.sync.dma_start(out=xT_t[:], in_=d_xT[:, nb * P:(nb + 1) * P])
            for k in range(2):
                ps_h = psA.tile([P, H], f32, tag="mm")
                nc.tensor.matmul(out=ps_h[:, 0:P], lhsT=D_sb[:, k * P:(k + 1) * P],
                                 rhs=xT_t[:], start=True, stop=True)
                nc.vector.tensor_scalar_add(hT(nb, k), ps_h[:, 0:P],
                                            base_col[:, k:k + 1])
                nc.vector.tensor_copy(out=hTb(nb, k), in_=hT(nb, k))
            emit_gemm(0, nb)
            if nb % 2 == 1:
                emit_ag_piece(0, nb // 2)
                if nb == 1:
                    a_prefetch(0)
                elif nb == 5:
                    a_prefetch(1)
                elif nb == 9:
                    a_prefetch(2)

        # ---- layers -----------------------------------------------------
        for l in range(L):
            # Table loads ride the gpsimd queue: each waits on its gather
            # piece, exactly the order the Comms engine completes them, so
            # nothing else ever queues behind a blocked trigger.
            for s in range(NSEG):
                nc.gpsimd.dma_start(
                    out=tab_sb[:, s * CPS * H:(s + 1) * CPS * H].rearrange(
                        "p (c w) -> p c w", c=NCORE),
                    in_=ag_outs[l][s][:].rearrange("(c p) w -> p c w", p=P))
            nc.gpsimd.memset(acc_sq[:], 0.0)
            # local mini-chains: the core's own 10 chunks consumed straight
            # from hws_sb while the AllGather pieces are still in flight
            for nb in range(BPC):
                ps_l = psA.tile([P, H], f32, tag="mm")
                for j in range(BPC // 2):
                    lhsT3 = aloc_sb[:, (nb * BPC + 2 * j) * P:
                                    (nb * BPC + 2 * j + 2) * P].rearrange(
                        "p (k m) -> p k m", k=2)
                    rhs3 = hws_sb[:, 2 * j * H:(2 * j + 2) * H].rearrange(
                        "p (k n) -> p k n", k=2)
                    nc.tensor.matmul(
                        out=ps_l[:], lhsT=lhsT3, rhs=rhs3,
                        start=(j == 0), stop=(j == BPC // 2 - 1), perf_mode=DR)
                nc.vector.tensor_copy(out=t_all[:, nb * H:(nb + 1) * H], in_=ps_l[:])
            for s in range(NSEG):
                if s < 2:
                    a_prefetch(s + 3)
                a_t = a_fifo.pop(0)
                for nb in range(BPC):
                    emit_seg_chain(a_t, nb, s)

            # stats: one cross-partition reduce, 32-row replicate, AllReduce
            ps_s = psB.tile([1, 2 * H], f32, tag="vec")
            nc.tensor.matmul(out=ps_s[:], lhsT=ones128[:], rhs=acc_sq[:],
                             start=True, stop=True)
            st_sb = scal[:, 6 * H:8 * H]
            nc.vector.tensor_copy(out=st_sb, in_=ps_s[:])
            st_rep = work.tile([RREP, 2 * H], f32, tag="strep", bufs=1)
            ps_r2 = psB.tile([RREP, 2 * H], f32, tag="vec")
            nc.tensor.matmul(out=ps_r2[:], lhsT=ones1[:, 0:RREP], rhs=st_sb,
                             start=True, stop=True)
            nc.vector.tensor_copy(out=st_rep[:], in_=ps_r2[:])
            nc.sync.dma_start(out=ar_in[:], in_=st_rep[:])
            nc.gpsimd.collective_compute(
                "AllReduce", OP.add, replica_groups=rg,
                ins=[ar_in[:]], outs=[ar_outs[l][:]])
            nc.sync.dma_start(out=stv[:], in_=ar_outs[l][0:1, :])

            # transpose t into tT while the AllReduce is in flight (TensorE
            # is otherwise idle in this window)
            for nb in range(BPC):
                for k in range(2):
                    ps_t2 = psB.tile([P, P], f32, tag="pst")
                    nc.tensor.transpose(
                        out=ps_t2[:], in_=t_all[:, nb * H + k * P:nb * H + (k + 1) * P],
                        identity=ident_f[:])
                    nc.vector.tensor_copy(out=tT(nb, k), in_=ps_t2[:])

            # BN scalar math on [1,256] rows: a = gamma*istd, c = beta - mu*a
            mu = scal[:, H:2 * H]
            var = scal[:, 2 * H:3 * H]
            msq = scal[:, 5 * H:6 * H]
            nc.vector.tensor_scalar_mul(mu, stv[:, 0:H], 1.0 / N)
            nc.vector.tensor_scalar_mul(var, stv[:, H:2 * H], 1.0 / N)
            nc.vector.tensor_tensor(out=msq, in0=mu, in1=mu, op=OP.mult)
            nc.vector.tensor_tensor(out=var, in0=var, in1=msq, op=OP.subtract)
            nc.vector.tensor_scalar_add(var, var, BN_EPS)
            nc.vector.reciprocal_approx_fast(out=var, in_=var)
            nc.scalar.activation(out=var, in_=var, func=FT.Sqrt)  # istd
            av = scal[:, 3 * H:4 * H]
            cv = scal[:, 4 * H:5 * H]
            nc.vector.tensor_tensor(out=av, in0=var,
                                    in1=gb_sb[:, l * H:(l + 1) * H], op=OP.mult)
            nc.vector.tensor_tensor(out=msq, in0=mu, in1=av, op=OP.mult)
            nc.vector.tensor_tensor(out=cv, in0=gb_sb[:, (L + l) * H:(L + l + 1) * H],
                                    in1=msq, op=OP.subtract)
            # column-ize a||c: 4 tiny matmuls [1,128]^T @ [1,1] -> [128,1]
            ps_col = psB.tile([P, 4], f32, tag="vec")
            for j in range(4):
                nc.tensor.matmul(out=ps_col[:, j:j + 1],
                                 lhsT=scal[:, 3 * H + j * P:3 * H + (j + 1) * P],
                                 rhs=ones1[:, 0:1], start=True, stop=True,
                                 skip_group_check=True)
            nc.vector.tensor_copy(out=bncol[:], in_=ps_col[:])

            # apply: h^T += relu(t^T * a + c) per (block, half); fused DVE
            # per-partition scalar op + Scalar-engine relu. Immediately GEMM
            # the updated block for the next layer; post the AllGather halves
            # as soon as each half's blocks are done.
            for nb in range(BPC):
                for k in range(2):
                    u = work.tile([P, P], f32, tag="tmp")
                    nc.vector.tensor_scalar(
                        out=u[:], in0=tT(nb, k),
                        scalar1=bncol[:, k:k + 1], scalar2=bncol[:, 2 + k:3 + k],
                        op0=OP.mult, op1=OP.add)
                    r = work.tile([P, P], f32, tag="tmp2")
                    nc.scalar.activation(out=r[:], in_=u[:], func=FT.Relu)
                    nc.vector.tensor_tensor(out=hT(nb, k), in0=hT(nb, k),
                                            in1=r[:], op=OP.add)
                    nc.vector.tensor_copy(out=hTb(nb, k), in_=hT(nb, k))
                if l < L - 1:
                    emit_gemm(l + 1, nb)
                    if nb % 2 == 1:
                        emit_ag_piece(l + 1, nb // 2)
                        if nb == 1:
                            a_prefetch(0)
                        elif nb == 5:
                            a_prefetch(1)
                        elif nb == 9:
                            a_prefetch(2)
                else:
                    # last layer: transpose back to node-major, pool matmuls,
                    # accumulate in SBUF (keeps PSUM banks free for chains)
                    hb_t = work.tile([P, H], bf16, tag="hb")
                    for k in range(2):
                        ps_tr = psB.tile([P, P], bf16, tag="pst")
                        nc.tensor.transpose(out=ps_tr[:], in_=hTb(nb, k),
                                            identity=ident_bf[:])
                        nc.vector.tensor_copy(out=hb_t[:, k * P:(k + 1) * P],
                                              in_=ps_tr[:])
                    pssl = psel_sb[:, nb * G:(nb + 1) * G]
                    for k in range(2):
                        ps_p = psB.tile([P, G], f32, tag="vec")
                        nc.tensor.matmul(out=ps_p[:], lhsT=hb_t[:, k * P:(k + 1) * P],
                                         rhs=pssl, start=True, stop=True)
                        nc.vector.tensor_tensor(
                            out=g_acc[:, k * G:(k + 1) * G],
                            in0=g_acc[:, k * G:(k + 1) * G], in1=ps_p[:], op=OP.add)

        # ---- pooling readout --------------------------------------------
        nc.sync.dma_start(out=pr_in[0:P, :], in_=g_acc[:, 0:G])
        nc.sync.dma_start(out=pr_in[P:2 * P, :], in_=g_acc[:, G:2 * G])
        nc.gpsimd.collective_compute(
            "AllReduce", OP.add, replica_groups=rg,
            ins=[pr_in[:]], outs=[pr_out[:]])
        g0 = work.tile([P, G], f32, tag="g0", bufs=1)
        g1 = work.tile([P, G], f32, tag="g1", bufs=1)
        nc.sync.dma_start(out=g0[:], in_=pr_out[0:P, :])
        nc.sync.dma_start(out=g1[:], in_=pr_out[P:2 * P, :])
        ps_r = psB.tile([P, G], f32, tag="vec")
        nc.tensor.matmul(out=ps_r[:], lhsT=ones1[:], rhs=rcnt_sb[:], start=True, stop=True)
        rc_rep = work.tile([P, G], f32, tag="rc_rep", bufs=1)
        nc.vector.tensor_copy(out=rc_rep[:], in_=ps_r[:])
        nc.vector.tensor_tensor(out=g0[:], in0=g0[:], in1=rc_rep[:], op=OP.mult)
        nc.vector.tensor_tensor(out=g1[:], in0=g1[:], in1=rc_rep[:], op=OP.mult)

        # MLP head (transposed: weights are lhsT, graphs along free dim)
        ps1 = psB.tile([P, G], f32, tag="vec")
        nc.tensor.matmul(out=ps1[:], lhsT=w1_sb[:, 0:P], rhs=g0[:], start=True, stop=False)
        nc.tensor.matmul(out=ps1[:], lhsT=w1_sb[:, P:2 * P], rhs=g1[:], start=False, stop=True)
        y1 = work.tile([P, G], f32, tag="y1", bufs=1)
        nc.scalar.activation(out=y1[:], in_=ps1[:], func=FT.Relu, bias=b1_sb[:, 0:1])
        ps2 = psB.tile([64, G], f32, tag="vec")
        nc.tensor.matmul(out=ps2[:], lhsT=w2_sb[:], rhs=y1[:], start=True, stop=True)
        y2 = work.tile([64, G], f32, tag="y2", bufs=1)
        nc.scalar.activation(out=y2[:], in_=ps2[:], func=FT.Relu, bias=b2_sb[:, 0:1])
        ps3 = psB.tile([1, G], f32, tag="vec")
        nc.tensor.matmul(out=ps3[:], lhsT=w3_sb[:], rhs=y2[:], start=True, stop=True)
        y3 = work.tile([1, G], f32, tag="y3", bufs=1)
        nc.vector.tensor_scalar_add(y3[:], ps3[:], b3_sb[0:1, 0:1])
        nc.sync.dma_start(out=d_out[:], in_=y3[:])

    nc.compile()
    return nc


# --------------------------------------------------------------------------
# entry point
# --------------------------------------------------------------------------

def kernel(x, edge_index, batch_ids, emb, W, b, gamma, beta,
           mlp_W1, mlp_b1, mlp_W2, mlp_b2, mlp_W3, mlp_b3,
           _trace=False, _trace_kwargs=None):
    # NB: reference BN subtracts the per-channel mean, so the additive bias b
    # cancels exactly and is not needed by the device program.
    in_maps = _preprocess(x, edge_index, batch_ids, emb, W, gamma, beta,
                          mlp_W1, mlp_b1, mlp_W2, mlp_b2, mlp_W3, mlp_b3)
    if "nc" not in _compiled:
        _compiled["nc"] = _build()
    nc = _compiled["nc"]
    kw = {}
    if _trace:
        kw = dict(trace=True, **(_trace_kwargs or {}))
    res = run_bass_kernel_spmd(nc, in_maps, core_ids=list(range(NCORE)), **kw)
    out = np.asarray(res.results[0]["out"], np.float32).reshape(G, 1)
    kernel._last_results = res
    return out


# revision 62
# speedup vs baseline: 1.1208x; 1.1074x over previous
"""Trainium2 Bass kernel for HIVNet GCN message passing (8-core SPMD).

v8 strategy (baseline 2.29ms -> v6 pure-dense 826us -> v7 DoubleRow 638us):
  - Pad N=10000 nodes to 10240 = 80 chunks x 128; core c owns 10 dst-blocks
    (global chunks c*10..c*10+9).
  - Per layer: hws = (h @ W[l])*nrm*32 on the owned shard, cast fp8e4m3,
    AllGather the partition-major table in two halves; aggregation is pure
    dense one-hot adjacency on TensorE using fp8 DoubleRow matmuls
    (both operands fp8, contraction 256/instruction, 2x bf16 throughput).
    The x32 table scale keeps hws out of fp8 subnormals; the dst-side norm
    carries the 1/32.
  - h lives TRANSPOSED (h^T: H on partitions, nodes on free dim):
      * the next-layer GEMM consumes h^T directly as lhsT (no transposes),
      * BN apply is one fused per-partition tensor_scalar (t^T*a + c) + relu,
      * t^T transposes run inside the BN AllReduce window (Tensor idle),
      * BN scale/shift column-ized via 4 tiny matmuls (no 128-row bcast).
  - BN stats: fused sum||sumsq reduce, 32-row replicate, Shared-output
    AllReduce; warmup AllGather at t=0 absorbs comms cold-start skew.
  - Readout: transpose h back per block (last layer only), one-hot pool
    matmuls accumulated in SBUF, 257-row AllReduce, redundant 3-layer MLP.
"""

import os
import sys

sys.path.insert(0, "/opt/trn_rl_repo")

from contextlib import ExitStack

import numpy as np
import ml_dtypes

from concourse import bass, mybir, bacc, tile, library_config
from concourse.bass_utils import run_bass_kernel_spmd
from concourse.masks import make_identity

NCORE = 8
P = 128
H = 256
L = 4
NF = 9
G = 256
N = 10000
BPC = 10                # dst blocks per core
NPC = BPC * P           # 1280 nodes per core
NPAD = NCORE * NPC      # 10240
NCHUNK = NPAD // P      # 80 src chunks
HB = BPC // 2           # blocks per AllGather half
BN_EPS = 1e-5
TSCALE = 32.0           # fp8 table scale

f32 = mybir.dt.float32
bf16 = mybir.dt.bfloat16
f8 = mybir.dt.float8e4
bfnp = ml_dtypes.bfloat16

FT = mybir.ActivationFunctionType
OP = mybir.AluOpType
DR = mybir.MatmulPerfMode.DoubleRow

_compiled = {}

NSEG = 5                # AllGather pieces per layer (2 blocks each)
BPS = BPC // NSEG       # blocks per gather piece
CPS = NCORE * BPS       # chunks per gather piece (16)

# chunk consumption order: fifth-major (blocks {2s,2s+1} of every core form
# gather piece s), so dense-chain segment s can start as soon as piece s
# lands; within a piece, core-major ascending = the gathered tab layout.
CHUNK_ORDER = [g for s in range(NSEG) for g in range(NCHUNK)
               if g % BPC in (2 * s, 2 * s + 1)]


# --------------------------------------------------------------------------
# host-side structural preprocessing
# --------------------------------------------------------------------------

def _preprocess(x, edge_index, batch_ids, emb, W, gamma, beta,
                mlp_W1, mlp_b1, mlp_W2, mlp_b2, mlp_W3, mlp_b3):
    src = np.asarray(edge_index[0], np.int64)
    dst = np.asarray(edge_index[1], np.int64)
    # self loops for every real node (weight nrm[d]^2 folds in)
    src_all = np.concatenate([src, np.arange(N, dtype=np.int64)])
    dst_all = np.concatenate([dst, np.arange(N, dtype=np.int64)])
    order = np.argsort(dst_all, kind="stable")
    s_sorted = src_all[order]
    d_sorted = dst_all[order]

    deg = np.bincount(dst_all, minlength=NPAD).astype(np.float64)  # incl self

    nblk = NCORE * BPC
    starts = np.searchsorted(d_sorted, np.arange(nblk) * P)
    ends = np.searchsorted(d_sorted, (np.arange(nblk) + 1) * P)

    # dense adjacency per dst block, chunk-major in CHUNK_ORDER.
    # The 10 chunks OWNED by the dst core are split out into A_local (kept
    # resident in SBUF, consumed from hws_sb before the AllGather lands) and
    # zeroed in the streamed A.
    A_blocks = {}
    A_local = {}
    for g in range(nblk):
        c, nb = divmod(g, BPC)
        e_s = s_sorted[starts[g]:ends[g]]
        e_d = d_sorted[starts[g]:ends[g]] - g * P
        A = np.zeros((NPAD, P), np.float32)
        np.add.at(A, (e_s, e_d), 1.0)
        A = A.reshape(NCHUNK, P, P)
        own = A[c * BPC:(c + 1) * BPC].copy()             # [10, P, P]
        A[c * BPC:(c + 1) * BPC] = 0.0
        A = A[CHUNK_ORDER]                                # reorder chunks
        # fp8 e4m3: edge multiplicities (<= 3 incl. self loop) are exact,
        # and fp8 x fp8 DoubleRow matmul runs at 2x bf16 throughput.
        A_blocks[(c, nb)] = np.ascontiguousarray(
            A.transpose(1, 0, 2).reshape(P, NCHUNK * P)
        ).astype(ml_dtypes.float8_e4m3)
        A_local[(c, nb)] = np.ascontiguousarray(
            own.transpose(1, 0, 2).reshape(P, BPC * P)
        ).astype(ml_dtypes.float8_e4m3)

    # graph pool one-hot [node, graph] (bf16: values 0/1 exact)
    bids = np.asarray(batch_ids, np.int64)
    psel_full = np.zeros((NPAD, G), np.float32)
    psel_full[np.arange(N), bids] = 1.0
    cnt = np.bincount(bids, minlength=G).astype(np.float64)
    rcnt = (1.0 / np.maximum(cnt, 1.0)).astype(np.float32)[None, :]

    x_np = np.zeros((NPAD, NF), np.float32)
    x_np[:N] = np.asarray(x, np.float64)

    Wf = np.asarray(W, np.float32)
    W_lhsT = Wf.reshape(L, 2, P, H).transpose(2, 0, 1, 3).reshape(P, L * 2 * H)
    gm = np.asarray(gamma, np.float32)
    bt = np.asarray(beta, np.float32)
    gb = np.concatenate([gm.reshape(-1), bt.reshape(-1)])[None, :]
    embf = np.asarray(emb, np.float32)
    emb0 = np.ascontiguousarray(embf[:, 0, :])
    emb1 = np.ascontiguousarray(embf[:, 1, :])
    w1 = np.asarray(mlp_W1, np.float32).reshape(2, P, P).transpose(1, 0, 2).reshape(P, 2 * P)
    w2 = np.asarray(mlp_W2, np.float32)
    w3 = np.asarray(mlp_W3, np.float32)
    b1 = np.asarray(mlp_b1, np.float32).reshape(P, 1)
    b2 = np.asarray(mlp_b2, np.float32).reshape(64, 1)
    b3 = np.asarray(mlp_b3, np.float32).reshape(1, 1)

    in_maps = []
    for c in range(NCORE):
        lo, hi = c * NPC, (c + 1) * NPC
        # fifth-major A tiles: tile s holds ALL 10 dst blocks' columns for
        # gather piece s (16 chunks each), block-major inside.
        Ab = np.stack([A_blocks[(c, nb)] for nb in range(BPC)], axis=1)
        Ac = Ab.reshape(P, BPC, NSEG, CPS * P).transpose(0, 2, 1, 3)
        Ac = np.ascontiguousarray(Ac).reshape(P, BPC * NCHUNK * P)

        degc = deg[lo:hi].reshape(BPC, P).T
        maskc = (degc > 0).astype(np.float32)
        degc = np.maximum(degc, 1.0).astype(np.float32)

        pselc = psel_full[lo:hi].reshape(BPC, P, G)
        pselc = np.ascontiguousarray(pselc.transpose(1, 0, 2)).reshape(P, BPC * G)

        Aloc = np.concatenate([A_local[(c, nb)] for nb in range(BPC)], axis=1)
        in_maps.append(dict(
            A=Ac, Aloc=Aloc, xT=np.ascontiguousarray(x_np[lo:hi].T),
            deg=degc, mask=maskc, psel=pselc.astype(bfnp),
            W=W_lhsT.astype(bfnp), gb=gb, emb0=emb0, emb1=emb1,
            w1=w1, w2=w2, w3=w3, b1=b1, b2=b2, b3=b3, rcnt=rcnt,
        ))
    return in_maps


# --------------------------------------------------------------------------
# device program
# --------------------------------------------------------------------------

def _build():
    nc = bacc.Bacc(None, target_bir_lowering=False)

    d_A = nc.dram_tensor("A", [P, BPC * NCHUNK * P], f8, kind="ExternalInput")
    d_Aloc = nc.dram_tensor("Aloc", [P, BPC * BPC * P], f8, kind="ExternalInput")
    d_xT = nc.dram_tensor("xT", [NF, NPC], f32, kind="ExternalInput")
    d_deg = nc.dram_tensor("deg", [P, BPC], f32, kind="ExternalInput")
    d_mask = nc.dram_tensor("mask", [P, BPC], f32, kind="ExternalInput")
    d_psel = nc.dram_tensor("psel", [P, BPC * G], bf16, kind="ExternalInput")
    d_W = nc.dram_tensor("W", [P, L * 2 * H], bf16, kind="ExternalInput")
    d_gb = nc.dram_tensor("gb", [1, 2 * L * H], f32, kind="ExternalInput")
    d_emb0 = nc.dram_tensor("emb0", [NF, H], f32, kind="ExternalInput")
    d_emb1 = nc.dram_tensor("emb1", [NF, H], f32, kind="ExternalInput")
    d_w1 = nc.dram_tensor("w1", [P, 2 * P], f32, kind="ExternalInput")
    d_w2 = nc.dram_tensor("w2", [P, 64], f32, kind="ExternalInput")
    d_w3 = nc.dram_tensor("w3", [64, 1], f32, kind="ExternalInput")
    d_b1 = nc.dram_tensor("b1", [P, 1], f32, kind="ExternalInput")
    d_b2 = nc.dram_tensor("b2", [64, 1], f32, kind="ExternalInput")
    d_b3 = nc.dram_tensor("b3", [1, 1], f32, kind="ExternalInput")
    d_rcnt = nc.dram_tensor("rcnt", [1, G], f32, kind="ExternalInput")
    d_out = nc.dram_tensor("out", [1, G], f32, kind="ExternalOutput")

    rg = [list(range(NCORE))]
    SW = BPS * H         # gather-piece payload width per partition (512 cols)

    with tile.TileContext(nc) as tc, ExitStack() as ctx:
        pers = ctx.enter_context(tc.tile_pool(name="pers", bufs=1))
        psA = ctx.enter_context(tc.tile_pool(name="psA", bufs=4, space="PSUM"))
        psB = ctx.enter_context(tc.tile_pool(name="psB", bufs=2, space="PSUM"))
        apool = ctx.enter_context(tc.tile_pool(name="apool", bufs=4))
        work = ctx.enter_context(tc.tile_pool(name="work", bufs=2))
        stream = ctx.enter_context(tc.tile_pool(name="stream", bufs=2))
        dram = ctx.enter_context(tc.tile_pool(name="dram", bufs=2, space="DRAM"))

        # ---- persistent SBUF state -------------------------------------
        deg_sb = pers.tile([P, BPC], f32, tag="deg")
        mask_sb = pers.tile([P, BPC], f32, tag="mask")
        psel_sb = pers.tile([P, BPC * G], bf16, tag="psel")
        W_sb = pers.tile([P, L * 2 * H], bf16, tag="W")
        gb_sb = pers.tile([1, 2 * L * H], f32, tag="gb")
        emb0_sb = pers.tile([NF, H], f32, tag="emb0")
        emb1_sb = pers.tile([NF, H], f32, tag="emb1")
        w1_sb = pers.tile([P, 2 * P], f32, tag="w1")
        w2_sb = pers.tile([P, 64], f32, tag="w2")
        w3_sb = pers.tile([64, 1], f32, tag="w3")
        b1_sb = pers.tile([P, 1], f32, tag="b1")
        b2_sb = pers.tile([64, 1], f32, tag="b2")
        b3_sb = pers.tile([1, 1], f32, tag="b3")

        tab_sb = pers.tile([P, NCHUNK * H], f8, tag="tab")
        hT_sb = pers.tile([P, BPC * 2 * P], f32, tag="hT")
        hTb_sb = pers.tile([P, BPC * 2 * P], bf16, tag="hTb")
        hws_sb = pers.tile([P, BPC * H], f8, tag="hws")
        t_all = pers.tile([P, BPC * H], f32, tag="t_all")
        tT_sb = pers.tile([P, BPC * 2 * P], f32, tag="tT")
        nrm_sb = pers.tile([P, BPC], f32, tag="nrm")
        nrm32_sb = pers.tile([P, BPC], f32, tag="nrm32")
        nrm32x_sb = pers.tile([P, BPC], f32, tag="nrm32x")
        acc_sq = pers.tile([P, 2 * H], f32, tag="acc_sq")
        D_sb = pers.tile([NF, H], f32, tag="D")
        base_col = pers.tile([P, 2], f32, tag="base_col")
        bncol = pers.tile([P, 4], f32, tag="bncol")
        g_acc = pers.tile([P, 2 * G], f32, tag="g_acc")
        ident_bf = pers.tile([P, P], bf16, tag="ident")
        ident_f = pers.tile([P, P], f32, tag="identf")
        ones9 = pers.tile([NF, 1], f32, tag="ones9")
        ones1 = pers.tile([1, P], f32, tag="ones1")
        ones128 = pers.tile([P, 1], f32, tag="ones128")
        stv = pers.tile([1, 2 * H], f32, tag="stv")
        rcnt_sb = pers.tile([1, G], f32, tag="rcnt")
        aloc_sb = pers.tile([P, BPC * BPC * P], f8, tag="aloc")
        scal = pers.tile([1, 8 * H], f32, tag="scal")

        # ---- DRAM bounce buffers ---------------------------------------
        # AllGather pieces: ag_in[s][p, :] = hws rows for blocks {2s,2s+1}
        # (512B fp8 contiguous run per partition; ag_out row c*128+p holds
        # core c's piece-run for partition p). Collective outputs are Shared
        # scratchpad (single-writer: one output tile per collective).
        ag_ins = [dram.tile([P, SW], f8, tag=f"ag_in{s}", name=f"ag_in{s}")
                  for s in range(NSEG)]
        ag_outs = [
            [dram.tile([NCORE * P, SW], f8, tag=f"ag_out{s}_{l}", bufs=1,
                       name=f"ag_out{s}_{l}", addr_space="Shared")
             for s in range(NSEG)]
            for l in range(L)
        ]
        RREP = 32            # BN stats replication rows (payload 64KB)
        ar_in = dram.tile([RREP, 2 * H], f32, tag="ar_in")
        ar_outs = [dram.tile([RREP, 2 * H], f32, tag=f"ar_out_{l}", bufs=1,
                             name=f"ar_out_{l}", addr_space="Shared")
                   for l in range(L)]
        pr_in = dram.tile([2 * P, G], f32, tag="pr_in")
        pr_out = dram.tile([2 * P, G], f32, tag="pr_out", bufs=1,
                           addr_space="Shared")
        warm_in = dram.tile([P, BPC], f32, tag="warm_in")
        warm_out = dram.tile([NCORE * P, BPC], f32, tag="warm_out", bufs=1,
                             addr_space="Shared")


        # warmup collective FIRST: absorbs the one-time comms boot +
        # core-arrival skew while the encoder runs. Collectives cannot read
        # IO tensors, so bounce a tiny staged input through Internal DRAM.
        nc.sync.dma_start(out=warm_in[:], in_=d_deg[:])
        nc.gpsimd.collective_compute(
            "AllGather", OP.bypass, replica_groups=rg,
            ins=[warm_in[:]], outs=[warm_out[:]])

        # ---- input loads ------------------------------------------------
        # Small early-needed tensors go on the sync queue ahead of the
        # encoder's xT loads; bulk tensors ride the scalar/gpsimd queues so
        # they delay neither the encoder DMAs nor the first AllGather bounce.
        for t, d in [(deg_sb, d_deg), (mask_sb, d_mask), (W_sb, d_W),
                     (gb_sb, d_gb), (emb0_sb, d_emb0), (emb1_sb, d_emb1)]:
            nc.sync.dma_start(out=t[:], in_=d[:])
        nc.scalar.dma_start(out=aloc_sb[:], in_=d_Aloc[:])
        for t, d in [(psel_sb, d_psel), (w1_sb, d_w1), (w2_sb, d_w2),
                     (w3_sb, d_w3), (b1_sb, d_b1), (b2_sb, d_b2),
                     (b3_sb, d_b3), (rcnt_sb, d_rcnt)]:
            nc.gpsimd.dma_start(out=t[:], in_=d[:])

        make_identity(nc, ident_bf[:])
        make_identity(nc, ident_f[:])
        nc.vector.memset(ones9[:], 1.0)
        nc.vector.memset(ones1[:], 1.0)
        nc.vector.memset(ones128[:], 1.0)
        nc.vector.memset(g_acc[:], 0.0)

        # nrm = rsqrt(deg) * mask ; the fp8 table is stored x32 (keeps hws
        # out of fp8e4m3 subnormals); the dst-side norm absorbs the 1/32
        rdeg = work.tile([P, BPC], f32, tag="rdeg", bufs=1)
        nc.vector.reciprocal(out=rdeg[:], in_=deg_sb[:])
        nc.scalar.activation(out=rdeg[:], in_=rdeg[:], func=FT.Sqrt)
        nc.vector.tensor_tensor(out=nrm_sb[:], in0=rdeg[:], in1=mask_sb[:], op=OP.mult)
        nc.vector.tensor_scalar_mul(nrm32_sb[:], nrm_sb[:], 1.0 / TSCALE)
        nc.vector.tensor_scalar_mul(nrm32x_sb[:], nrm_sb[:], TSCALE)

        # encoder prep: D = emb1 - emb0 ; base columns b_k = emb0_half_k^T @ 1
        nc.vector.tensor_tensor(out=D_sb[:], in0=emb1_sb[:], in1=emb0_sb[:], op=OP.subtract)
        for k in range(2):
            ps_b = psB.tile([P, 1], f32, tag="vec")
            nc.tensor.matmul(out=ps_b[:], lhsT=emb0_sb[:, k * P:(k + 1) * P],
                             rhs=ones9[:], start=True, stop=True)
            nc.vector.tensor_copy(out=base_col[:, k:k + 1], in_=ps_b[:])

        def hT(nb, k):
            return hT_sb[:, (nb * 2 + k) * P:(nb * 2 + k + 1) * P]

        def hTb(nb, k):
            return hTb_sb[:, (nb * 2 + k) * P:(nb * 2 + k + 1) * P]

        def tT(nb, k):
            return tT_sb[:, (nb * 2 + k) * P:(nb * 2 + k + 1) * P]

        def emit_gemm(l, nb):
            """hws[nb] = (h @ W[l]) * nrm * 32, fp8. lhsT is h^T directly."""
            ps_g = psA.tile([P, H], f32, tag="mm")
            for k in range(2):
                nc.tensor.matmul(
                    out=ps_g[:], lhsT=hTb(nb, k),
                    rhs=W_sb[:, (l * 2 + k) * H:(l * 2 + k + 1) * H],
                    start=(k == 0), stop=(k == 1))
            nc.vector.tensor_scalar_mul(hws_sb[:, nb * H:(nb + 1) * H],
                                        ps_g[:], nrm32x_sb[:, nb:nb + 1])

        def emit_ag_piece(l, s):
            nc.sync.dma_start(out=ag_ins[s][:],
                              in_=hws_sb[:, s * SW:(s + 1) * SW])
            nc.gpsimd.collective_compute(
                "AllGather", OP.bypass, replica_groups=rg,
                ins=[ag_ins[s][:]], outs=[ag_outs[l][s][:]])

        a_fifo = []

        def a_prefetch(s):
            # one fifth-tile: all 10 dst blocks x 16 chunks of gather piece s
            # (2.6MB). On the scalar queue so the ag_in bounces on sync are
            # never stuck behind a bulk transfer.
            a_t = apool.tile([P, BPC * CPS * P], f8, tag="A")
            nc.scalar.dma_start(
                out=a_t[:], in_=d_A[:, s * BPC * CPS * P:(s + 1) * BPC * CPS * P])
            a_fifo.append(a_t)

        # Aggregation runs in NSEG segments; segment s consumes gather piece
        # s for all 10 dst blocks, in chunk PAIRS via fp8 DoubleRow matmuls
        # (contraction 256/instruction at 2 fp8 rows/cycle).
        def emit_seg_chain(a_t, nb, s):
            ps_t = psA.tile([P, H], f32, tag="mm")
            for t in range(CPS // 2):
                lhsT3 = a_t[:, (nb * CPS + 2 * t) * P:
                            (nb * CPS + 2 * t + 2) * P].rearrange(
                    "p (j m) -> p j m", j=2)
                rhs3 = tab_sb[:, (s * CPS + 2 * t) * H:
                              (s * CPS + 2 * t + 2) * H].rearrange(
                    "p (j n) -> p j n", j=2)
                nc.tensor.matmul(
                    out=ps_t[:], lhsT=lhsT3, rhs=rhs3,
                    start=(t == 0), stop=(t == CPS // 2 - 1), perf_mode=DR)
            tsl = t_all[:, nb * H:(nb + 1) * H]
            nc.vector.tensor_tensor(out=tsl, in0=tsl, in1=ps_t[:], op=OP.add)
            if s == NSEG - 1:
                # t = (nrm/32)*sum; BN stats accumulate on the idle GpSimd
                # engine so the DVE queue never gates the stats matmul
                nc.vector.tensor_scalar_mul(tsl, tsl, nrm32_sb[:, nb:nb + 1])
                sq = work.tile([P, H], f32, tag="tmp3")
                nc.gpsimd.tensor_tensor(out=sq[:], in0=tsl, in1=tsl, op=OP.mult)
                nc.gpsimd.tensor_tensor(out=acc_sq[:, 0:H], in0=acc_sq[:, 0:H],
                                        in1=tsl, op=OP.add)
                nc.gpsimd.tensor_tensor(out=acc_sq[:, H:2 * H],
                                        in0=acc_sq[:, H:2 * H], in1=sq[:], op=OP.add)

        # encoder: h0^T = D^T x^T + base (directly transposed)
        for nb in range(BPC):
            xT_t = stream.tile([NF, P], f32, tag="xT_t")
            nc.sync.dma_start(out=xT_t[:], in_=d_xT[:, nb * P:(nb + 1) * P])
            for k in range(2):
                ps_h = psA.tile([P, H], f32, tag="mm")
                nc.tensor.matmul(out=ps_h[:, 0:P], lhsT=D_sb[:, k * P:(k + 1) * P],
                                 rhs=xT_t[:], start=True, stop=True)
                nc.vector.tensor_scalar_add(hT(nb, k), ps_h[:, 0:P],
                                            base_col[:, k:k + 1])
                nc.vector.tensor_copy(out=hTb(nb, k), in_=hT(nb, k))
            emit_gemm(0, nb)
            if nb % 2 == 1:
                emit_ag_piece(0, nb // 2)
                if nb == 1:
                    a_prefetch(0)
                elif nb == 5:
                    a_prefetch(1)
                elif nb == 9:
                    a_prefetch(2)

        # ---- layers -----------------------------------------------------
        for l in range(L):
            # Table loads ride the gpsimd queue: each waits on its gather
            # piece, exactly the order the Comms engine completes them, so
            # nothing else ever queues behind a blocked trigger.
            for s in range(NSEG):
                nc.gpsimd.dma_start(
                    out=tab_sb[:, s * CPS * H:(s + 1) * CPS * H].rearrange(
                        "p (c w) -> p c w", c=NCORE),
                    in_=ag_outs[l][s][:].rearrange("(c p) w -> p c w", p=P))
            nc.gpsimd.memset(acc_sq[:], 0.0)
            # local mini-chains: the core's own 10 chunks consumed straight
            # from hws_sb while the AllGather pieces are still in flight
            for nb in range(BPC):
                ps_l = psA.tile([P, H], f32, tag="mm")
                for j in range(BPC // 2):
                    lhsT3 = aloc_sb[:, (nb * BPC + 2 * j) * P:
                                    (nb * BPC + 2 * j + 2) * P].rearrange(
                        "p (k m) -> p k m", k=2)
                    rhs3 = hws_sb[:, 2 * j * H:(2 * j + 2) * H].rearrange(
                        "p (k n) -> p k n", k=2)
                    nc.tensor.matmul(
                        out=ps_l[:], lhsT=lhsT3, rhs=rhs3,
                        start=(j == 0), stop=(j == BPC // 2 - 1), perf_mode=DR)
                nc.vector.tensor_copy(out=t_all[:, nb * H:(nb + 1) * H], in_=ps_l[:])
            for s in range(NSEG):
                if s < 2:
                    a_prefetch(s + 3)
                a_t = a_fifo.pop(0)
                for nb in range(BPC):
                    emit_seg_chain(a_t, nb, s)

            # stats: one cross-partition reduce, 32-row replicate, AllReduce
            ps_s = psB.tile([1, 2 * H], f32, tag="vec")
            nc.tensor.matmul(out=ps_s[:], lhsT=ones128[:], rhs=acc_sq[:],
                             start=True, stop=True)
            st_sb = scal[:, 6 * H:8 * H]
            nc.vector.tensor_copy(out=st_sb, in_=ps_s[:])
            st_rep = work.tile([RREP, 2 * H], f32, tag="strep", bufs=1)
            ps_r2 = psB.tile([RREP, 2 * H], f32, tag="vec")
            nc.tensor.matmul(out=ps_r2[:], lhsT=ones1[:, 0:RREP], rhs=st_sb,
                             start=True, stop=True)
            nc.vector.tensor_copy(out=st_rep[:], in_=ps_r2[:])
            nc.sync.dma_start(out=ar_in[:], in_=st_rep[:])
            nc.gpsimd.collective_compute(
                "AllReduce", OP.add, replica_groups=rg,
                ins=[ar_in[:]], outs=[ar_outs[l][:]])
            nc.sync.dma_start(out=stv[:], in_=ar_outs[l][0:1, :])

            # transpose t into tT while the AllReduce is in flight (TensorE
            # is otherwise idle in this window)
            for nb in range(BPC):
                for k in range(2):
                    ps_t2 = psB.tile([P, P], f32, tag="pst")
                    nc.tensor.transpose(
                        out=ps_t2[:], in_=t_all[:, nb * H + k * P:nb * H + (k + 1) * P],
                        identity=ident_f[:])
                    nc.vector.tensor_copy(out=tT(nb, k), in_=ps_t2[:])

            # BN scalar math on [1,256] rows: a = gamma*istd, c = beta - mu*a
            mu = scal[:, H:2 * H]
            var = scal[:, 2 * H:3 * H]
            msq = scal[:, 5 * H:6 * H]
            nc.vector.tensor_scalar_mul(mu, stv[:, 0:H], 1.0 / N)
            nc.vector.tensor_scalar_mul(var, stv[:, H:2 * H], 1.0 / N)
            nc.vector.tensor_tensor(out=msq, in0=mu, in1=mu, op=OP.mult)
            nc.vector.tensor_tensor(out=var, in0=var, in1=msq, op=OP.subtract)
            nc.vector.tensor_scalar_add(var, var, BN_EPS)
            nc.vector.reciprocal_approx_fast(out=var, in_=var)
            nc.scalar.activation(out=var, in_=var, func=FT.Sqrt)  # istd
            av = scal[:, 3 * H:4 * H]
            cv = scal[:, 4 * H:5 * H]
            nc.vector.tensor_tensor(out=av, in0=var,
                                    in1=gb_sb[:, l * H:(l + 1) * H], op=OP.mult)
            nc.vector.tensor_tensor(out=msq, in0=mu, in1=av, op=OP.mult)
            nc.vector.tensor_tensor(out=cv, in0=gb_sb[:, (L + l) * H:(L + l + 1) * H],
                                    in1=msq, op=OP.subtract)
            # column-ize a||c: 4 tiny matmuls [1,128]^T @ [1,1] -> [128,1]
            ps_col = psB.tile([P, 4], f32, tag="vec")
            for j in range(4):
                nc.tensor.matmul(out=ps_col[:, j:j + 1],
                                 lhsT=scal[:, 3 * H + j * P:3 * H + (j + 1) * P],
                                 rhs=ones1[:, 0:1], start=True, stop=True,
                                 skip_group_check=True)
            nc.vector.tensor_copy(out=bncol[:], in_=ps_col[:])

            # apply: h^T += relu(t^T * a + c) per (block, half); fused DVE
            # per-partition scalar op + Scalar-engine relu. Immediately GEMM
            # the updated block for the next layer; post the AllGather halves
            # as soon as each half's blocks are done.
            for nb in range(BPC):
                for k in range(2):
                    u = work.tile([P, P], f32, tag="tmp")
                    nc.vector.tensor_scalar(
                        out=u[:], in0=tT(nb, k),
                        scalar1=bncol[:, k:k + 1], scalar2=bncol[:, 2 + k:3 + k],
                        op0=OP.mult, op1=OP.add)
                    r = work.tile([P, P], f32, tag="tmp2")
                    nc.scalar.activation(out=r[:], in_=u[:], func=FT.Relu)
                    nc.vector.tensor_tensor(out=hT(nb, k), in0=hT(nb, k),
                                            in1=r[:], op=OP.add)
                    nc.vector.tensor_copy(out=hTb(nb, k), in_=hT(nb, k))
                if l < L - 1:
                    emit_gemm(l + 1, nb)
                    if nb % 2 == 1:
                        emit_ag_piece(l + 1, nb // 2)
                        if nb == 1:
                            a_prefetch(0)
                        elif nb == 5:
                            a_prefetch(1)
                        elif nb == 9:
                            a_prefetch(2)
                else:
                    # last layer: transpose back to node-major, pool matmuls,
                    # accumulate in SBUF (keeps PSUM banks free for chains)
                    hb_t = work.tile([P, H], bf16, tag="hb")
                    for k in range(2):
                        ps_tr = psB.tile([P, P], bf16, tag="pst")
                        nc.tensor.transpose(out=ps_tr[:], in_=hTb(nb, k),
                                            identity=ident_bf[:])
                        nc.vector.tensor_copy(out=hb_t[:, k * P:(k + 1) * P],
                                              in_=ps_tr[:])
                    pssl = psel_sb[:, nb * G:(nb + 1) * G]
                    for k in range(2):
                        ps_p = psB.tile([P, G], f32, tag="vec")
                        nc.tensor.matmul(out=ps_p[:], lhsT=hb_t[:, k * P:(k + 1) * P],
                                         rhs=pssl, start=True, stop=True)
                        nc.vector.tensor_tensor(
                            out=g_acc[:, k * G:(k + 1) * G],
                            in0=g_acc[:, k * G:(k + 1) * G], in1=ps_p[:], op=OP.add)

        # ---- pooling readout --------------------------------------------
        nc.sync.dma_start(out=pr_in[0:P, :], in_=g_acc[:, 0:G])
        nc.sync.dma_start(out=pr_in[P:2 * P, :], in_=g_acc[:, G:2 * G])
        nc.gpsimd.collective_compute(
            "AllReduce", OP.add, replica_groups=rg,
            ins=[pr_in[:]], outs=[pr_out[:]])
        g0 = work.tile([P, G], f32, tag="g0", bufs=1)
        g1 = work.tile([P, G], f32, tag="g1", bufs=1)
        nc.sync.dma_start(out=g0[:], in_=pr_out[0:P, :])
        nc.sync.dma_start(out=g1[:], in_=pr_out[P:2 * P, :])
        ps_r = psB.tile([P, G], f32, tag="vec")
        nc.tensor.matmul(out=ps_r[:], lhsT=ones1[:], rhs=rcnt_sb[:], start=True, stop=True)
        rc_rep = work.tile([P, G], f32, tag="rc_rep", bufs=1)
        nc.vector.tensor_copy(out=rc_rep[:], in_=ps_r[:])
        nc.vector.tensor_tensor(out=g0[:], in0=g0[:], in1=rc_rep[:], op=OP.mult)
        nc.vector.tensor_tensor(out=g1[:], in0=g1[:], in1=rc_rep[:], op=OP.mult)

        # MLP head (transposed: weights are lhsT, graphs along free dim)
        ps1 = psB.tile([P, G], f32, tag="vec")
        nc.tensor.matmul(out=ps1[:], lhsT=w1_sb[:, 0:P], rhs=g0[:], start=True, stop=False)
        nc.tensor.matmul(out=ps1[:], lhsT=w1_sb[:, P:2 * P], rhs=g1[:], start=False, stop=True)
        y1 = work.tile([P, G], f32, tag="y1", bufs=1)
        nc.scalar.activation(out=y1[:], in_=ps1[:], func=FT.Relu, bias=b1_sb[:, 0:1])
        ps2 = psB.tile([64, G], f32, tag="vec")
        nc.tensor.matmul(out=ps2[:], lhsT=w2_sb[:], rhs=y1[:], start=True, stop=True)
        y2 = work.tile([64, G], f32, tag="y2", bufs=1)
        nc.scalar.activation(out=y2[:], in_=ps2[:], func=FT.Relu, bias=b2_sb[:, 0:1])
        ps3 = psB.tile([1, G], f32, tag="vec")
        nc.tensor.matmul(out=ps3[:], lhsT=w3_sb[:], rhs=y2[:], start=True, stop=True)
        y3 = work.tile([1, G], f32, tag="y3", bufs=1)
        nc.vector.tensor_scalar_add(y3[:], ps3[:], b3_sb[0:1, 0:1])
        nc.sync.dma_start(out=d_out[:], in_=y3[:])

    nc.compile()
    return nc


# --------------------------------------------------------------------------
# entry point
# --------------------------------------------------------------------------

def kernel(x, edge_index, batch_ids, emb, W, b, gamma, beta,
           mlp_W1, mlp_b1, mlp_W2, mlp_b2, mlp_W3, mlp_b3,
           _trace=False, _trace_kwargs=None):
    # NB: reference BN subtracts the per-channel mean, so the additive bias b
    # cancels exactly and is not needed by the device program.
    in_maps = _preprocess(x, edge_index, batch_ids, emb, W, gamma, beta,
                          mlp_W1, mlp_b1, mlp_W2, mlp_b2, mlp_W3, mlp_b3)
    if "nc" not in _compiled:
        _compiled["nc"] = _build()
    nc = _compiled["nc"]
    kw = {}
    if _trace:
        kw = dict(trace=True, **(_trace_kwargs or {}))
    res = run_bass_kernel_spmd(nc, in_maps, core_ids=list(range(NCORE)), **kw)
    out = np.asarray(res.results[0]["out"], np.float32).reshape(G, 1)
    kernel._last_results = res
    return out
